# revision 48
# baseline (speedup 1.0000x reference)
"""Trainium2 Bass kernel for a DehazeBlock:
    res1 = relu(conv3x3(x, w1) + b1) + x
    res2 = conv3x3(res1, w2) + b2
    out  = deform_conv(res2, p_w, p_b, dw) + x

Sharding: 8 cores = 4 batch x 2 H-halves (32 rows each, data-parallel,
communication-free; each core gets a zero-padded 40-row input slab).

Deform strategy (all-PE, no gathers): bilinear sampling at (base + t),
|t| < 1, decomposes over a 3-tap stencil (relu(-t), 1-|t|, relu(t)).
Fold the per-tap 256x256 channel mix first (y_n = DW_n @ res2), then
out[m,u] += sum_v y_n[v,m] * B_n[v,u] with B_n banded, built on-chip by
gpsimd local_scatter (host-precomputed index table, 64-stride output
space) from F*G weight planes staged through DRAM skew round-trips.

Schedule: conv1 -> conv2 with the offset conv (fp8 DoubleRow, K=256 per
instruction) interleaved; F/G staging, skewed reads, sw multiplies, swT
transposes and the serial gpsimd scatter chain all launch mid-conv2 so
the chain finishes long before the banded pass consumes it -> 19
y-tiles -> one full-width banded pass into 8 single-bank PSUM tiles
seeded with the x-residual by identity matmuls, each bank copied +
DMA'd out as its accumulation group closes.
"""

import os
import numpy as np
import ml_dtypes

import bass_rust
import concourse.bass as bass
import concourse.mybir as mybir
import concourse.tile as tile
from concourse import bacc
from concourse.bass_utils import run_bass_kernel_spmd
from concourse.masks import make_identity

bf16 = ml_dtypes.bfloat16
f8 = ml_dtypes.float8_e4m3
F32 = mybir.dt.float32
BF = mybir.dt.bfloat16
F8 = mybir.dt.float8e4
I16 = mybir.dt.int16

P = 128
CB = 2              # channel blocks (256 = 2*128)
W = 66              # padded row width
TS = 40             # x slab rows
R1 = 38             # res1 rows
R2 = 36             # res2 rows (= v rows)
RO = 32             # output rows per core
U = RO * W          # 2112 output pixel space
XL = TS * W + 2     # 2642 padded flat x row-span (+1 lead, +1 tail elem)
R1L = R1 * W + 2    # 2510
VT = 19             # v tiles
VPW = VT * P        # 2432
FW = 2720           # staging row width for F/G planes
MARG = 266
BW = 262            # banded-matrix u-window width per (tap, vtile)
TAPS = 9
N_CORES = 8
B5BUFS = 13         # b5 ring size; vtiles >= B5BUFS scattered during banded
PW_SCALE = 1024.0   # fp8 prescale for the tiny offset-conv weights
# 3-group staging pipeline: (vtile range, v0, vn, write ulo, write uhi)
SGROUPS = [(0, 6, 0, 768, 0, 12 * W),
           (6, 12, 768, 768, 6 * W, 24 * W),
           (12, 19, 1536, 896, 18 * W, 32 * W)]

CONV1_CHUNKS = [(0, 7), (7, 7), (14, 7), (21, 7), (28, 7), (35, 3)]
CONV2_CHUNKS = [(0, 6), (6, 6), (12, 6), (18, 6), (24, 6), (30, 6)]
OFFS_CHUNKS = [(0, 6), (6, 6), (12, 6), (18, 6), (24, 6), (30, 2)]
BW64 = 256          # banded window width in 64-stride output space
U64 = 2048          # 32 rows x 64 cols
DR = mybir.MatmulPerfMode.DoubleRow


def _u64_map(vt, n, a, b, p):
    """Output position (64-stride) hit by source row v=128vt+p, slot
    (n,a,b); None if it lands in a pad column / outside the image."""
    nr, ncc = n // 3 - 1, n % 3 - 1
    u66 = 128 * vt + p - 68 - 66 * nr - ncc - 66 * a + b
    if not (0 <= u66 < 2112 and 1 <= (u66 % 66) <= 64):
        return None
    return 64 * (u66 // 66) + (u66 % 66) - 1


def _w064_table():
    tab = {}
    for vt in range(VT):
        for n in range(TAPS):
            us = [u for a in range(3) for b in range(4) for p in range(P)
                  if (u := _u64_map(vt, n, a, b, p)) is not None]
            tab[(vt, n)] = min(us) if us else None
    return tab


_CACHE = {}
LAST_RESULTS = None
W064 = _w064_table()

MIN = mybir.AluOpType.min
MAX = mybir.AluOpType.max


def _mk_src(t, dims, off):
    s = t.ap().copy()
    s.ap = bass_rust.VecI64Pair(dims)
    s.offset = off
    return s


def _split_at_banks(lo, hi, base):
    """Split [lo, hi) (psum-tile-relative) at 512-elem bank boundaries."""
    segs = []
    a = lo
    while a < hi:
        b = min(hi, ((a - base) // 512 + 1) * 512 + base)
        segs.append((a, b))
        a = b
    return segs


def _build_program():
    nc = bacc.Bacc("TRN2", target_bir_lowering=False, debug=False,
                   num_devices=N_CORES)

    # ---------------- dram I/O ----------------
    xsb_d = nc.dram_tensor("xsb", [CB, P, XL], BF, kind="ExternalInput")
    maskr_d = nc.dram_tensor("maskr", [P, TS], BF, kind="ExternalInput")
    w1t_d = nc.dram_tensor("w1t", [CB, TAPS, P, 256], BF, kind="ExternalInput")
    w2t_d = nc.dram_tensor("w2t", [CB, TAPS, P, 256], BF, kind="ExternalInput")
    pw8_d = nc.dram_tensor("pw8", [TAPS, P, 96], F8, kind="ExternalInput")
    dwt_d = nc.dram_tensor("dwt", [CB, P, TAPS * 256], BF, kind="ExternalInput")
    b1_d = nc.dram_tensor("b1", [CB, P, 1], F32, kind="ExternalInput")
    b2_d = nc.dram_tensor("b2", [CB, P, 1], F32, kind="ExternalInput")
    pb_d = nc.dram_tensor("pb", [41, 1], F32, kind="ExternalInput")
    permm_d = nc.dram_tensor("permm", [108, 108], BF, kind="ExternalInput")
    xs64_d = nc.dram_tensor("xs64", [CB, P, U64], BF, kind="ExternalInput")
    idx_d = nc.dram_tensor("idx", [P, VT * 108], I16, kind="ExternalInput")
    out_d = nc.dram_tensor("out", [CB, P, RO, 64], F32, kind="ExternalOutput")

    fdramH = [nc.dram_tensor(f"fdram{h}", [27, FW], BF) for h in range(3)]
    gdramH = [nc.dram_tensor(f"gdram{h}", [36, FW], BF) for h in range(3)]

    RELU = mybir.ActivationFunctionType.Relu
    ABS = mybir.ActivationFunctionType.Abs
    IDENT = mybir.ActivationFunctionType.Identity
    MUL = mybir.AluOpType.mult
    ADD = mybir.AluOpType.add

    with tile.TileContext(nc) as tc:
      with tc.tile_pool(name="perm", bufs=1) as perm:
        ident = perm.tile([P, P], BF, name="ident")
        xs64 = [perm.tile([P, U64], BF, name=f"xs64_{c}") for c in range(CB)]
        idxt = perm.tile([P, VT * 108], I16, name="idxt")
        maskr = perm.tile([P, TS], BF, name="maskr")
        b1 = [perm.tile([P, 1], F32, name=f"b1{c}") for c in range(CB)]
        b2 = [perm.tile([P, 1], F32, name=f"b2{c}") for c in range(CB)]
        pb = perm.tile([41, 1], F32, name="pb")
        zb = perm.tile([P, 1], F32, name="zb")
        permm = perm.tile([108, 108], BF, name="permm")

        make_identity(nc, ident[:])
        nc.vector.memset(zb[:], 0)

        with tc.tile_pool(name="swpool", bufs=1) as swpool:
          ys, b5s, swTs = [], [], []
          swG = [swpool.tile([108, g[3]], BF, name=f"sw{i}")
                 for i, g in enumerate(SGROUPS)]
          dwt = [swpool.tile([P, TAPS * 256], BF, name=f"dwt{c}")
                 for c in range(CB)]
          res2b = [swpool.tile([P, VPW], BF, name=f"res2b{c}")
                   for c in range(CB)]
          with tc.tile_pool(name="prtpct", bufs=1) as prtpct:
            prtG = [prtpct.tile([108, g[3]], BF, name=f"prt{i}")
                    for i, g in enumerate(SGROUPS)]
            pctG = [prtpct.tile([108, g[3]], BF, name=f"pct{i}")
                    for i, g in enumerate(SGROUPS)]
            with tc.tile_pool(name="swtpool", bufs=1) as swtpool, \
                 tc.tile_pool(name="b5apool", bufs=1) as b5apool, \
                 tc.tile_pool(name="b5bpool", bufs=1) as b5bpool, \
                 tc.tile_pool(name="outpool", bufs=1) as outpool:

                # pre-create every ring tile so they sit BELOW the 'early'
                # region on the allocation stack (they outlive it)
                swT_t = [swtpool.tile([P, 108], BF, name=f"swT{i}")
                         for i in range(VT)]
                b5a_t = [b5apool.tile([P, 5 * BW64], BF, name=f"b5a{i}")
                         for i in range(B5BUFS)]
                b5b_t = [b5bpool.tile([P, 4 * BW64], BF, name=f"b5b{i}")
                         for i in range(B5BUFS)]
                outt_t = [outpool.tile([P, 512], F32, name=f"outt{i}")
                          for i in range(4)]

                def emit_swmul(gi):
                    # vector: Pool tensor_tensor is 2.5x slower and forces
                    # ucode lib swaps against the local_scatter chain
                    nc.vector.tensor_tensor(swG[gi][:], prtG[gi][:],
                                            pctG[gi][:], MUL)

                def emit_b5_scat(vt):
                    swT = swT_t[vt]
                    i0 = 108 * vt
                    b5a = b5a_t[vt % B5BUFS]
                    b5b = b5b_t[vt % B5BUFS]
                    nc.gpsimd.local_scatter(b5a[:], swT[:, 0:60],
                                            idxt[:, i0:i0 + 60],
                                            channels=P,
                                            num_elems=5 * BW64,
                                            num_idxs=60)
                    nc.gpsimd.local_scatter(b5b[:], swT[:, 60:108],
                                            idxt[:, i0 + 60:i0 + 108],
                                            channels=P,
                                            num_elems=4 * BW64,
                                            num_idxs=48)
                    b5s.append((b5a, b5b))

                tpsum_cm = tc.tile_pool(name="tpsum", bufs=1, space="PSUM")
                tpsum = tpsum_cm.__enter__()
                pst_t = [tpsum.tile([P, 108], F32, name=f"pst{i}")
                         for i in range(2)]

                def emit_b5_trans(vt, psy=None):
                    v0 = vt * P
                    gi = 0 if vt < 6 else (1 if vt < 12 else 2)
                    gv0 = SGROUPS[gi][2]
                    swsrc = swG[gi][:, v0 - gv0:v0 - gv0 + P]
                    if psy is None:
                        pst = pst_t[vt % 2][:, 0:108]
                    else:
                        pst = psy.tile([P, 512], F32, tag="psy",
                                       name=f"psty{vt}")[:, 0:108]
                    nc.tensor.matmul(pst, swsrc, permm[:],
                                     start=True, stop=True)
                    swT = swT_t[vt]
                    nc.scalar.activation(swT[:], pst, IDENT,
                                         bias=zb[:], scale=1.0)

                with tc.tile_pool(name="early", bufs=1) as early, \
                     tc.tile_pool(name="chunk", bufs=3) as chunk:
                    xsb = [early.tile([P, XL], BF, name=f"xsb{c}")
                           for c in range(CB)]
                    w1t = [early.tile([P, TAPS * 256], BF, name=f"w1t{c}")
                           for c in range(CB)]
                    w2t = [early.tile([P, TAPS * 256], BF, name=f"w2t{c}")
                           for c in range(CB)]
                    pwt8 = early.tile([P, TAPS * 96], F8, name="pwt8")
                    res1b = [early.tile([P, R1L], BF, name=f"res1b{c}")
                             for c in range(CB)]
                    zsb = early.tile([36, FW], BF, name="zsb")
                    res2f8 = early.tile([P, 2 * VPW], F8, name="res2f8")
                    fsb = early.tile([96, U], BF, name="fsb")
                    gsb = early.tile([96, U], BF, name="gsb")

                    # -------- input DMAs (merged, priority order) --------
                    # xsb[0] split so the first conv1 matmuls start early
                    XH = 1 + 22 * W
                    nc.sync.dma_start(out=xsb[0][:, 0:XH],
                                      in_=xsb_d.ap()[0][:, 0:XH])
                    nc.sync.dma_start(
                        out=w1t[0][:, 0:256], in_=w1t_d.ap()[0, 0])
                    nc.sync.dma_start(out=xsb[0][:, XH:XL],
                                      in_=xsb_d.ap()[0][:, XH:XL])
                    for t in range(1, TAPS):
                        nc.sync.dma_start(
                            out=w1t[0][:, t * 256:(t + 1) * 256],
                            in_=w1t_d.ap()[0, t])
                    # cb1 slab + weights on the idle gpsimd queue so they
                    # stream in parallel with the cb0 critical loads
                    nc.gpsimd.dma_start(out=xsb[1][:, 0:XH],
                                        in_=xsb_d.ap()[1][:, 0:XH])
                    nc.gpsimd.dma_start(out=xsb[1][:, XH:XL],
                                        in_=xsb_d.ap()[1][:, XH:XL])
                    for t in range(TAPS):
                        nc.gpsimd.dma_start(
                            out=w1t[1][:, t * 256:(t + 1) * 256],
                            in_=w1t_d.ap()[1, t])
                    nc.sync.dma_start(out=maskr[:], in_=maskr_d.ap())
                    for c in range(CB):
                        nc.sync.dma_start(out=b1[c][:], in_=b1_d.ap()[c])

                    def load_w(dram, tile_, c, mout):
                        src = _mk_src(dram,
                                      [[mout, P], [P * mout, TAPS], [1, mout]],
                                      c * TAPS * P * mout)
                        nc.sync.dma_start(out=tile_[:], in_=src)

                    for c in range(CB):
                        load_w(w2t_d, w2t[c], c, 256)
                        nc.sync.dma_start(out=b2[c][:], in_=b2_d.ap()[c])
                    for t in range(TAPS):
                        nc.sync.dma_start(
                            out=pwt8[:, t * 96:(t + 1) * 96],
                            in_=pw8_d.ap()[t])
                    nc.sync.dma_start(out=pb[:], in_=pb_d.ap())
                    nc.sync.dma_start(out=permm[:], in_=permm_d.ap())
                    nc.sync.dma_start(out=idxt[:], in_=idx_d.ap())
                    for c in range(CB):
                        nc.sync.dma_start(out=xs64[c][:], in_=xs64_d.ap()[c])
                    for c in range(CB):
                        nc.sync.dma_start(out=dwt[c][:], in_=dwt_d.ap()[c])

                    # zero staging planes (margins must read as 0)
                    nc.vector.memset(zsb[:], 0)
                    for h in range(3):
                        nc.sync.dma_start(out=fdramH[h].ap(), in_=zsb[:27, :])
                        nc.sync.dma_start(out=gdramH[h].ap(), in_=zsb[:, :])

                    # pre-zero so conv posts can write image columns only
                    # (pad columns stay zero; no end-of-phase memset barrier)
                    for c in range(CB):
                        nc.vector.memset(res1b[c][:], 0)
                        nc.vector.memset(res2b[c][:], 0)
                    nc.vector.memset(res2f8[:], 0)

                    # ---- phase 1: conv1 -> res1b ----
                    with tc.tile_pool(name="c1psum", bufs=6,
                                      space="PSUM") as cpsum:
                        for mb in range(CB):
                            psums = [cpsum.tile([P, 462], F32, tag="c1ps",
                                                name=f"c1ps_{mb}_{i}")
                                     for i in range(len(CONV1_CHUNKS))]
                            for cb in range(CB):
                                for t in range(TAPS):
                                    ky, kx = t // 3, t % 3
                                    lhsT = w1t[cb][:, t * 256 + mb * P:
                                                   t * 256 + mb * P + P]
                                    first = (cb == 0 and t == 0)
                                    last = (cb == CB - 1 and t == TAPS - 1)
                                    for ci, (c0, cr) in enumerate(CONV1_CHUNKS):
                                        o = 1 + (c0 + ky) * W + kx - 1
                                        nc.tensor.matmul(
                                            psums[ci][:, :cr * W], lhsT,
                                            xsb[cb][:, o:o + cr * W],
                                            start=first, stop=last)
                            for ci, (c0, cr) in enumerate(CONV1_CHUNKS):
                                n = cr * W
                                tmp = chunk.tile([P, 462], BF, tag="post")
                                nc.scalar.activation(tmp[:, :n],
                                                     psums[ci][:, :n],
                                                     RELU, bias=b1[mb][:],
                                                     scale=1.0)
                                tmp2 = chunk.tile([P, 462], BF, tag="post2")
                                nc.vector.tensor_tensor(
                                    tmp2[:, :n], tmp[:, :n],
                                    xsb[mb][:, 1 + (c0 + 1) * W:
                                            1 + (c0 + 1) * W + n], ADD)
                                mv = maskr[:, c0 + 1:c0 + 1 + cr, None] \
                                    .to_broadcast((P, cr, 64))
                                ov = res1b[mb][:, 1 + c0 * W:
                                               1 + (c0 + cr) * W]
                                nc.vector.tensor_tensor(
                                    ov.rearrange("p (r w) -> p r w",
                                                 w=W)[:, :, 1:65],
                                    tmp2[:, :n].rearrange(
                                        "p (r w) -> p r w", w=W)[:, :, 1:65],
                                    mv, MUL)

                    # ---- phase 2+3: conv2, offset conv, staging, scatters
                    def emit_conv2_chunk(mb, ps, ci):
                        e0, cr = CONV2_CHUNKS[ci]
                        for cb in range(CB):
                            for t in range(TAPS):
                                ky, kx = t // 3, t % 3
                                lhsT = w2t[cb][:, t * 256 + mb * P:
                                               t * 256 + mb * P + P]
                                first = (cb == 0 and t == 0)
                                last = (cb == CB - 1 and t == TAPS - 1)
                                o = 1 + (e0 + ky) * W + kx - 1
                                nc.tensor.matmul(
                                    ps[:, :cr * W], lhsT,
                                    res1b[cb][:, o:o + cr * W],
                                    start=first, stop=last)
                        n = cr * W
                        tmp = chunk.tile([P, 462], BF, tag="post")
                        nc.scalar.activation(tmp[:, :n], ps[:, :n],
                                             IDENT, bias=b2[mb][:], scale=1.0)
                        mv = maskr[:, e0 + 2:e0 + 2 + cr, None] \
                            .to_broadcast((P, cr, 64))
                        ov = res2b[mb][:, e0 * W:(e0 + cr) * W]
                        nc.vector.tensor_tensor(
                            ov.rearrange("p (r w) -> p r w", w=W)[:, :, 1:65],
                            tmp[:, :n].rearrange("p (r w) -> p r w",
                                                 w=W)[:, :, 1:65],
                            mv, MUL)
                        # fp8 copy for the offset conv rhs: second masked
                        # MUL straight from tmp (pads pre-zeroed at head)
                        of8 = res2f8[:, mb * VPW + e0 * W:
                                     mb * VPW + (e0 + cr) * W]
                        nc.vector.tensor_tensor(
                            of8.rearrange("p (r w) -> p r w", w=W)[:, :, 1:65],
                            tmp[:, :n].rearrange("p (r w) -> p r w",
                                                 w=W)[:, :, 1:65],
                            mv, MUL)

                    def emit_offs_chunk(opsum, m):
                        i0, cr = OFFS_CHUNKS[m]
                        n = cr * W
                        # fp8 DoubleRow: rhs windows must start 2B-aligned,
                        # so keep rhs at a fixed even base per tap and shift
                        # the psum write window by 1-kx instead (edge cols
                        # land on discarded pad positions).
                        ops = opsum.tile([48, 400], F32, tag="ops")
                        r3 = res2f8[:, :].rearrange("p (i v) -> p i v", i=2)
                        for t in range(TAPS):
                            ky, kx = t // 3, t % 3
                            lhsT = pwt8[:, t * 96:(t + 1) * 96].rearrange(
                                "p (i m) -> p i m", i=2)
                            r0 = (i0 + 1 + ky) * W
                            jj0 = 3 - kx
                            nc.tensor.matmul(
                                ops[:, jj0:jj0 + n + 1], lhsT,
                                r3[:, :, r0:r0 + n + 1],
                                start=(t == 0), stop=(t == TAPS - 1),
                                perf_mode=DR)
                        u0 = i0 * W
                        offs_c = chunk.tile([41, 396], F32, tag="offsc")
                        nc.scalar.activation(offs_c[:, :n], ops[0:41, 2:2 + n],
                                             IDENT, bias=pb[:],
                                             scale=1.0 / PW_SCALE)
                        offc_c = offs_c[32:41, :]
                        tmpa = chunk.tile([9, 396], BF, tag="tmpac")
                        tmpb = chunk.tile([9, 396], BF, tag="tmpbc")
                        # F rows: 0..8 relu(-t_r), 32.. 1-|t_r|, 64.. relu(t_r)
                        nc.vector.tensor_scalar(fsb[0:9, u0:u0 + n],
                                                offs_c[0:9, :n],
                                                0.0, -1.0, MIN, MUL)
                        nc.scalar.activation(tmpa[:, :n], offs_c[0:9, :n], ABS)
                        nc.vector.tensor_scalar(fsb[32:41, u0:u0 + n],
                                                tmpa[:, :n],
                                                -1.0, 1.0, MUL, ADD)
                        nc.vector.tensor_scalar(fsb[64:73, u0:u0 + n],
                                                offs_c[0:9, :n],
                                                0.0, 0.0, MAX, ADD)
                        # G rows: 0..8 relu(t_c) (jc=1), 32.. 1-|t_c| (jc=2),
                        # 64.. relu(-t_c) (jc=3)
                        nc.vector.tensor_scalar(gsb[0:9, u0:u0 + n],
                                                offc_c[:, :n],
                                                0.0, 0.0, MAX, ADD)
                        nc.scalar.activation(tmpb[:, :n], offc_c[:, :n], ABS)
                        nc.vector.tensor_scalar(gsb[32:41, u0:u0 + n],
                                                tmpb[:, :n],
                                                -1.0, 1.0, MUL, ADD)
                        nc.vector.tensor_scalar(gsb[64:73, u0:u0 + n],
                                                offc_c[:, :n],
                                                0.0, -1.0, MIN, MUL)

                    def emit_staging_group(gi):
                        v0, vn, ulo, uhi = SGROUPS[gi][2], SGROUPS[gi][3], \
                            SGROUPS[gi][4], SGROUPS[gi][5]
                        nu = uhi - ulo
                        # F skew: row 3n+a, col MARG+66*inr+inc+66a+u
                        for a in range(3):
                            dst = _mk_src(
                                fdramH[gi],
                                [[9 * FW + 66, 3], [3 * FW + 1, 3], [1, nu]],
                                a * FW + MARG + 66 * a + ulo)
                            nc.sync.dma_start(
                                out=dst, in_=fsb[32 * a:32 * a + 9, ulo:uhi])
                        # G: row 4n+jc, col MARG+66inr+inc+(3-jc)+u
                        # on sync: the scalar queue's activation backlog
                        # delayed these ~8us when they were scalar-issued
                        for jc in (1, 2, 3):
                            g0 = 32 * (jc - 1)
                            dst = _mk_src(
                                gdramH[gi],
                                [[12 * FW + 66, 3], [4 * FW + 1, 3], [1, nu]],
                                jc * FW + MARG + (3 - jc) + ulo)
                            nc.sync.dma_start(
                                out=dst, in_=gsb[g0:g0 + 9, ulo:uhi])
                        # reads: rows (a,n,b)/(a,n,c)-major, contiguous
                        rq = nc.gpsimd if gi == 0 else nc.sync
                        for a in range(3):
                            src = _mk_src(fdramH[gi],
                                          [[3 * FW, 9], [1, 4], [1, vn]],
                                          a * FW + MARG - 1 + v0)
                            rq.dma_start(out=prtG[gi][36 * a:36 * a + 36, :],
                                         in_=src)
                            src = _mk_src(gdramH[gi], [[FW, 36], [1, vn]],
                                          MARG + 2 - 66 * a + v0)
                            rq.dma_start(out=pctG[gi][36 * a:36 * a + 36, :],
                                         in_=src)
                        emit_swmul(gi)

                    NCH = len(OFFS_CHUNKS)
                    with tc.tile_pool(name="c2psum", bufs=3,
                                      space="PSUM") as c2psum, \
                         tc.tile_pool(name="opsum", bufs=1,
                                      space="PSUM") as opsum:
                        # alternate mb0/mb1 per row band: both channel
                        # blocks of each band finish together, so the
                        # offset conv / staging / scatter chain launches
                        # ~30us earlier than an mb-major sweep
                        for ci in range(len(CONV2_CHUNKS)):
                            for mb in range(CB):
                                ps = c2psum.tile([P, 462], F32, tag="c2ps",
                                                 name=f"c2ps_{mb}_{ci}")
                                emit_conv2_chunk(mb, ps, ci)
                            if ci >= 1:
                                emit_offs_chunk(opsum, ci - 1)
                            if ci == 2:
                                emit_staging_group(0)
                            elif ci == 3:
                                for vt in range(0, 6):
                                    emit_b5_trans(vt)
                            elif ci == 4:
                                emit_staging_group(1)
                            elif ci == 5:
                                for vt in range(6, 12):
                                    emit_b5_trans(vt)
                        emit_offs_chunk(opsum, NCH - 1)
                        emit_staging_group(2)
                        # scatters last on the Pool queue: all staged
                        # reads + sw multiplies are already queued ahead
                        for vt in range(0, 12):
                            emit_b5_scat(vt)

                # 'early' freed here; y / banded phases follow
                tpsum_cm.__exit__(None, None, None)
                ypool_cm = tc.tile_pool(name="ypool", bufs=VT)
                ypool = ypool_cm.__enter__()

                def emit_y(psy, vt):
                    v0 = vt * P
                    y = ypool.tile([P, TAPS * 256], BF, name=f"y{vt}",
                                   tag="y")
                    for pc in range(5):
                        a0 = pc * 512
                        a1 = min(a0 + 512, TAPS * 256)
                        ps = psy.tile([P, 512], F32, tag="psy")
                        for cb in range(CB):
                            nc.tensor.matmul(ps[:, :a1 - a0],
                                             res2b[cb][:, v0:v0 + P],
                                             dwt[cb][:, a0:a1],
                                             start=(cb == 0),
                                             stop=(cb == CB - 1))
                        if pc in (0, 2, 4):
                            nc.vector.tensor_copy(y[:, a0:a1],
                                                  ps[:, :a1 - a0])
                        else:
                            nc.scalar.activation(y[:, a0:a1],
                                                 ps[:, :a1 - a0],
                                                 IDENT, bias=zb[:],
                                                 scale=1.0)
                    ys.append(y)

                # ---- phases 4+5: y-builds and a bank-split banded pass.
                # Low-u half (banks 0-1) runs right after y0-10 so the
                # vt0-5 b5 buffers free early, unblocking scat13-18 while
                # y11-18 still build; high-u half then runs un-paced.
                mms = []
                for vt in range(VT):
                    for n in range(TAPS):
                        w0 = W064[(vt, n)]
                        if w0 is None:
                            continue
                        lo = max(w0, 0)
                        hi = min(w0 + BW64, U64)
                        mms.append((vt, n, w0, lo, hi))
                lastj = {}
                for j, (vt, n, w0, lo, hi) in enumerate(mms):
                    for (sa, sb_) in _split_at_banks(lo, hi, 0):
                        lastj[sa // 512] = j
                psbs = {}

                def emit_seeds(bpsum, banks):
                    for mb in range(CB):
                        for k in banks:
                            psbs[(mb, k)] = bpsum.tile(
                                [P, 512], F32, name=f"psb_{mb}_{k}")
                            nc.tensor.matmul(
                                psbs[(mb, k)][:, :512],
                                ident[:],
                                xs64[mb][:, 512 * k:512 * (k + 1)],
                                start=True, stop=False,
                                skip_group_check=True)

                def emit_mms(banks, jlo=0, jhi=None):
                    if jhi is None:
                        jhi = len(mms)
                    for j in range(jlo, jhi):
                        (vt, n, w0, lo, hi) = mms[j]
                        segs = [s for s in _split_at_banks(lo, hi, 0)
                                if s[0] // 512 in banks]
                        if not segs:
                            continue
                        y = ys[vt]
                        b5a, b5b = b5s[vt]
                        for mb in range(CB):
                            lhsT = y[:, n * 256 + mb * P:
                                     n * 256 + mb * P + P]
                            for (sa, sb_) in segs:
                                bk = sa // 512
                                c0 = sa - bk * 512
                                if n < 5:
                                    rhs = b5a[:, n * BW64 + sa - w0:
                                              n * BW64 + sb_ - w0]
                                else:
                                    rhs = b5b[:,
                                              (n - 5) * BW64 + sa - w0:
                                              (n - 5) * BW64 + sb_ - w0]
                                nc.tensor.matmul(
                                    psbs[(mb, bk)][:, c0:c0 + sb_ - sa],
                                    lhsT, rhs, start=False,
                                    stop=(j == lastj[bk]),
                                    skip_group_check=True)

                def emit_out(banks):
                    for bank in banks:
                        for mb in range(CB):
                            outt = outt_t[2 * (bank % 2) + mb]
                            if mb == 0:
                                nc.scalar.activation(
                                    outt[:, :],
                                    psbs[(mb, bank)][:, :],
                                    IDENT, bias=zb[:], scale=1.0)
                            else:
                                nc.vector.tensor_copy(
                                    outt[:, :],
                                    psbs[(mb, bank)][:, :])
                            nc.sync.dma_start(
                                out=out_d.ap()[mb, :, 8 * bank:
                                               8 * bank + 8, :],
                                in_=outt[:, :].rearrange(
                                    "p (r w) -> p r w", w=64))

                with tc.tile_pool(name="psy", bufs=6,
                                  space="PSUM") as psy:
                    emit_y(psy, 0)
                    emit_y(psy, 1)
                    for vt in range(12, VT):
                        emit_b5_trans(vt, psy=psy)
                    emit_b5_scat(12)
                    for vt in range(2, VT):
                        emit_y(psy, vt)

                with tc.tile_pool(name="bpsum", bufs=1,
                                  space="PSUM") as bpsum:
                    emit_seeds(bpsum, (0, 1, 2, 3))
                    jsplit = next(j for j, m in enumerate(mms)
                                  if m[0] >= 6)
                    emit_mms((0, 1, 2, 3), 0, jsplit)
                    for vt in range(B5BUFS, VT):
                        emit_b5_scat(vt)
                    emit_mms((0, 1, 2, 3), jsplit, None)
                    emit_out((0, 1, 2, 3))

                ypool_cm.__exit__(None, None, None)

    nc.finalize()
    return nc


def _pack_inputs(x, w1, b1, w2, b2, p_w, p_b, dw):
    """Build the 8 per-core input maps (numpy only)."""
    x = np.asarray(x, np.float32)

    def pack_w(w, mout):
        w = np.asarray(w, np.float32)
        out = np.empty((CB, TAPS, P, mout), bf16)
        for cb in range(CB):
            for t in range(TAPS):
                out[cb, t] = w[:, cb * P:(cb + 1) * P,
                               t // 3, t % 3].T.astype(bf16)
        return out

    w1t = pack_w(w1, 256)
    w2t = pack_w(w2, 256)
    # fp8 DoubleRow offset-conv weights: [TAPS, P, (cb, 41)] * PW_SCALE
    pwf = np.asarray(p_w, np.float32) * PW_SCALE
    pw8 = np.zeros((TAPS, P, 96), f8)
    for t in range(TAPS):
        for i in range(CB):
            blk = pwf[:, i * P:(i + 1) * P, t // 3, t % 3]  # [18, 128]
            pw8[t, :, i * 48 + 0:i * 48 + 9] = blk[0:9].T.astype(f8)
            pw8[t, :, i * 48 + 32:i * 48 + 41] = blk[9:18].T.astype(f8)
    dwt = np.empty((CB, P, TAPS * 256), bf16)
    dwf = np.asarray(dw, np.float32)
    for cb in range(CB):
        for t in range(TAPS):
            dwt[cb, :, t * 256:(t + 1) * 256] = \
                dwf[:, cb * P:(cb + 1) * P, t // 3, t % 3].T.astype(bf16)
    b1p = np.ascontiguousarray(np.asarray(b1, np.float32).reshape(CB, P, 1))
    b2p = np.ascontiguousarray(np.asarray(b2, np.float32).reshape(CB, P, 1))
    pb18 = np.asarray(p_b, np.float32).reshape(18)
    pbp = np.zeros((41, 1), np.float32)
    pbp[0:9, 0] = pb18[0:9]
    pbp[32:41, 0] = pb18[9:18]

    permm = np.zeros((108, 108), bf16)
    for n in range(9):
        for a in range(3):
            for b in range(4):
                permm[36 * a + 4 * n + b, 12 * n + 4 * a + b] = 1.0
    idx = np.full((P, VT * 108), -1, np.int16)
    for vt in range(VT):
        for n in range(TAPS):
            w0 = W064[(vt, n)]
            if w0 is None:
                continue
            zb_ = (n if n < 5 else n - 5) * BW64
            for a in range(3):
                for b in range(4):
                    l = 12 * n + 4 * a + b
                    for p in range(P):
                        u = _u64_map(vt, n, a, b, p)
                        if u is not None:
                            idx[p, 108 * vt + l] = zb_ + u - w0

    maps = []
    for core in range(N_CORES):
        b, half = core // 2, core % 2
        r0 = 32 * half
        slab = np.zeros((CB, P, TS, W), np.float32)
        g0, g1 = max(0, r0 - 4), min(64, r0 + 36)
        t0 = g0 - (r0 - 4)
        for cb in range(CB):
            slab[cb, :, t0:t0 + (g1 - g0), 1:65] = \
                x[b, cb * P:(cb + 1) * P, g0:g1, :]
        xsv = np.zeros((CB, P, XL), np.float32)
        xsv[:, :, 1:1 + TS * W] = slab.reshape(CB, P, TS * W)
        maskr = np.zeros((P, TS), bf16)
        valid = np.array([1.0 if 0 <= r0 - 4 + t < 64 else 0.0
                          for t in range(TS)], np.float32)
        maskr[:] = valid.astype(bf16)[None, :]
        xs64 = np.zeros((CB, P, U64), bf16)
        for cb in range(CB):
            xs64[cb] = slab[cb, :, 4:36, 1:65].reshape(P, U64).astype(bf16)
        maps.append({
            "xsb": xsv.astype(bf16), "maskr": maskr, "xs64": xs64,
            "w1t": w1t, "w2t": w2t, "pw8": pw8, "dwt": dwt,
            "b1": b1p, "b2": b2p, "pb": pbp, "permm": permm, "idx": idx,
        })
    return maps


def get_program():
    if "nc" not in _CACHE:
        _CACHE["nc"] = _build_program()
    return _CACHE["nc"]


def _ensure_ntff_hook():
    """The image's antenv lacks axon_hooks; inject a shim and register the
    NTFF profiling hook so trace=True works under axon."""
    import sys, types
    import antenv
    if "antenv.axon_hooks" in sys.modules:
        return
    mod = types.ModuleType("antenv.axon_hooks")
    mod._hook = None
    def set_axon_ntff_profile_hook(h):
        mod._hook = h
    def get_axon_ntff_profile_hook():
        return mod._hook
    mod.set_axon_ntff_profile_hook = set_axon_ntff_profile_hook
    mod.get_axon_ntff_profile_hook = get_axon_ntff_profile_hook
    sys.modules["antenv.axon_hooks"] = mod
    antenv.axon_hooks = mod
    try:
        from trn_agent_boot.trn_boot import _ntff_profile_via_ctypes
        hook = _ntff_profile_via_ctypes("/opt/axon/libaxon_pjrt.so")
        if hook is not None:
            set_axon_ntff_profile_hook(hook)
    except Exception as e:
        print("ntff hook setup failed:", e)


def kernel(x, w1, b1, w2, b2, p_w, p_b, dw):
    global LAST_RESULTS
    nc = get_program()
    maps = _pack_inputs(x, w1, b1, w2, b2, p_w, p_b, dw)
    trace = os.environ.get("DEHAZE_TRACE") == "1"
    if trace:
        _ensure_ntff_hook()
    res = run_bass_kernel_spmd(nc, maps, core_ids=list(range(N_CORES)),
                               trace=trace)
    LAST_RESULTS = res
    out = np.empty((4, 256, 64, 64), np.float32)
    for core in range(N_CORES):
        b, half = core // 2, core % 2
        o = res.results[core]["out"]  # [CB, P, RO, 64]
        out[b, :, 32 * half:32 * half + 32, :] = o.reshape(256, 32, 64)
    return out


# revision 49
# speedup vs baseline: 1.0168x; 1.0168x over previous
"""Trainium2 Bass kernel for a DehazeBlock:
    res1 = relu(conv3x3(x, w1) + b1) + x
    res2 = conv3x3(res1, w2) + b2
    out  = deform_conv(res2, p_w, p_b, dw) + x

Sharding: 8 cores = 4 batch x 2 H-halves (32 rows each, data-parallel,
communication-free; each core gets a zero-padded 40-row input slab).

Deform strategy (all-PE, no gathers): bilinear sampling at (base + t),
|t| < 1, decomposes over a 3-tap stencil (relu(-t), 1-|t|, relu(t)).
Fold the per-tap 256x256 channel mix first (y_n = DW_n @ res2), then
out[m,u] += sum_v y_n[v,m] * B_n[v,u] with B_n banded, built on-chip by
gpsimd local_scatter (host-precomputed index table, 64-stride output
space) from F*G weight planes staged through DRAM skew round-trips.

Schedule: conv1 -> conv2 with the offset conv (fp8 DoubleRow, K=256 per
instruction) interleaved; F/G staging, skewed reads, sw multiplies, swT
transposes and the serial gpsimd scatter chain all launch mid-conv2 so
the chain finishes long before the banded pass consumes it -> 19
y-tiles -> one full-width banded pass into 8 single-bank PSUM tiles
seeded with the x-residual by identity matmuls, each bank copied +
DMA'd out as its accumulation group closes.
"""

import os
import numpy as np
import ml_dtypes

import bass_rust
import concourse.bass as bass
import concourse.mybir as mybir
import concourse.tile as tile
from concourse import bacc
from concourse.bass_utils import run_bass_kernel_spmd
from concourse.masks import make_identity

bf16 = ml_dtypes.bfloat16
f8 = ml_dtypes.float8_e4m3
F32 = mybir.dt.float32
BF = mybir.dt.bfloat16
F8 = mybir.dt.float8e4
I16 = mybir.dt.int16

P = 128
CB = 2              # channel blocks (256 = 2*128)
W = 66              # padded row width
TS = 40             # x slab rows
R1 = 38             # res1 rows
R2 = 36             # res2 rows (= v rows)
RO = 32             # output rows per core
U = RO * W          # 2112 output pixel space
XL = TS * W + 2     # 2642 padded flat x row-span (+1 lead, +1 tail elem)
R1L = R1 * W + 2    # 2510
VT = 19             # v tiles
VPW = VT * P        # 2432
FW = 2720           # staging row width for F/G planes
MARG = 266
BW = 262            # banded-matrix u-window width per (tap, vtile)
TAPS = 9
N_CORES = 8
B5BUFS = 13         # b5 ring size; vtiles >= B5BUFS scattered during banded
PW_SCALE = 1024.0   # fp8 prescale for the tiny offset-conv weights
# 3-group staging pipeline: (vtile range, v0, vn, write ulo, write uhi)
SGROUPS = [(0, 6, 0, 768, 0, 12 * W),
           (6, 12, 768, 768, 6 * W, 24 * W),
           (12, 19, 1536, 896, 18 * W, 32 * W)]

CONV1_CHUNKS = [(0, 7), (7, 7), (14, 7), (21, 7), (28, 7), (35, 3)]
CONV2_CHUNKS = [(0, 6), (6, 6), (12, 6), (18, 6), (24, 6), (30, 6)]
OFFS_CHUNKS = [(0, 6), (6, 6), (12, 6), (18, 6), (24, 6), (30, 2)]
BW64 = 256          # banded window width in 64-stride output space
U64 = 2048          # 32 rows x 64 cols
DR = mybir.MatmulPerfMode.DoubleRow


def _u64_map(vt, n, a, b, p):
    """Output position (64-stride) hit by source row v=128vt+p, slot
    (n,a,b); None if it lands in a pad column / outside the image."""
    nr, ncc = n // 3 - 1, n % 3 - 1
    u66 = 128 * vt + p - 68 - 66 * nr - ncc - 66 * a + b
    if not (0 <= u66 < 2112 and 1 <= (u66 % 66) <= 64):
        return None
    return 64 * (u66 // 66) + (u66 % 66) - 1


def _w064_table():
    tab = {}
    for vt in range(VT):
        for n in range(TAPS):
            us = [u for a in range(3) for b in range(4) for p in range(P)
                  if (u := _u64_map(vt, n, a, b, p)) is not None]
            tab[(vt, n)] = min(us) if us else None
    return tab


_CACHE = {}
LAST_RESULTS = None
W064 = _w064_table()

MIN = mybir.AluOpType.min
MAX = mybir.AluOpType.max


def _mk_src(t, dims, off):
    s = t.ap().copy()
    s.ap = bass_rust.VecI64Pair(dims)
    s.offset = off
    return s


def _split_at_banks(lo, hi, base):
    """Split [lo, hi) (psum-tile-relative) at 512-elem bank boundaries."""
    segs = []
    a = lo
    while a < hi:
        b = min(hi, ((a - base) // 512 + 1) * 512 + base)
        segs.append((a, b))
        a = b
    return segs


def _build_program():
    nc = bacc.Bacc("TRN2", target_bir_lowering=False, debug=False,
                   num_devices=N_CORES)

    # ---------------- dram I/O ----------------
    xsb_d = nc.dram_tensor("xsb", [CB, P, XL], BF, kind="ExternalInput")
    maskr_d = nc.dram_tensor("maskr", [P, TS], BF, kind="ExternalInput")
    w1t_d = nc.dram_tensor("w1t", [CB, TAPS, P, 256], BF, kind="ExternalInput")
    w2t_d = nc.dram_tensor("w2t", [CB, TAPS, P, 256], BF, kind="ExternalInput")
    pw8_d = nc.dram_tensor("pw8", [TAPS, P, 96], F8, kind="ExternalInput")
    dwt_d = nc.dram_tensor("dwt", [CB, P, TAPS * 256], BF, kind="ExternalInput")
    b1_d = nc.dram_tensor("b1", [CB, P, 1], F32, kind="ExternalInput")
    b2_d = nc.dram_tensor("b2", [CB, P, 1], F32, kind="ExternalInput")
    pb_d = nc.dram_tensor("pb", [41, 1], F32, kind="ExternalInput")
    permm_d = nc.dram_tensor("permm", [108, 108], BF, kind="ExternalInput")
    xs64_d = nc.dram_tensor("xs64", [CB, P, U64], BF, kind="ExternalInput")
    idx_d = nc.dram_tensor("idx", [P, VT * 108], I16, kind="ExternalInput")
    out_d = nc.dram_tensor("out", [CB, P, RO, 64], F32, kind="ExternalOutput")

    fdramH = [nc.dram_tensor(f"fdram{h}", [27, FW], BF) for h in range(3)]
    gdramH = [nc.dram_tensor(f"gdram{h}", [36, FW], BF) for h in range(3)]

    RELU = mybir.ActivationFunctionType.Relu
    ABS = mybir.ActivationFunctionType.Abs
    IDENT = mybir.ActivationFunctionType.Identity
    MUL = mybir.AluOpType.mult
    ADD = mybir.AluOpType.add

    with tile.TileContext(nc) as tc:
      with tc.tile_pool(name="perm", bufs=1) as perm:
        ident = perm.tile([P, P], BF, name="ident")
        xs64 = [perm.tile([P, U64], BF, name=f"xs64_{c}") for c in range(CB)]
        idxt = perm.tile([P, VT * 108], I16, name="idxt")
        maskr = perm.tile([P, TS], BF, name="maskr")
        b1 = [perm.tile([P, 1], F32, name=f"b1{c}") for c in range(CB)]
        b2 = [perm.tile([P, 1], F32, name=f"b2{c}") for c in range(CB)]
        pb = perm.tile([41, 1], F32, name="pb")
        zb = perm.tile([P, 1], F32, name="zb")
        permm = perm.tile([108, 108], BF, name="permm")

        make_identity(nc, ident[:])
        nc.vector.memset(zb[:], 0)

        with tc.tile_pool(name="swpool", bufs=1) as swpool:
          ys, b5s, swTs = [], [], []
          swG = [swpool.tile([108, g[3]], BF, name=f"sw{i}")
                 for i, g in enumerate(SGROUPS)]
          dwt = [swpool.tile([P, TAPS * 256], BF, name=f"dwt{c}")
                 for c in range(CB)]
          res2b = [swpool.tile([P, VPW], BF, name=f"res2b{c}")
                   for c in range(CB)]
          with tc.tile_pool(name="prtpct", bufs=1) as prtpct:
            prtG = [prtpct.tile([108, g[3]], BF, name=f"prt{i}")
                    for i, g in enumerate(SGROUPS)]
            pctG = [prtpct.tile([108, g[3]], BF, name=f"pct{i}")
                    for i, g in enumerate(SGROUPS)]
            with tc.tile_pool(name="swtpool", bufs=1) as swtpool, \
                 tc.tile_pool(name="b5apool", bufs=1) as b5apool, \
                 tc.tile_pool(name="b5bpool", bufs=1) as b5bpool, \
                 tc.tile_pool(name="outpool", bufs=1) as outpool:

                # pre-create every ring tile so they sit BELOW the 'early'
                # region on the allocation stack (they outlive it)
                swT_t = [swtpool.tile([P, 108], BF, name=f"swT{i}")
                         for i in range(VT)]
                b5a_t = [b5apool.tile([P, 5 * BW64], BF, name=f"b5a{i}")
                         for i in range(B5BUFS)]
                b5b_t = [b5bpool.tile([P, 4 * BW64], BF, name=f"b5b{i}")
                         for i in range(B5BUFS)]
                outt_t = [outpool.tile([P, 512], F32, name=f"outt{i}")
                          for i in range(4)]

                def emit_swmul(gi):
                    # vector: Pool tensor_tensor is 2.5x slower and forces
                    # ucode lib swaps against the local_scatter chain
                    nc.vector.tensor_tensor(swG[gi][:], prtG[gi][:],
                                            pctG[gi][:], MUL)

                def emit_b5_scat(vt):
                    swT = swT_t[vt]
                    i0 = 108 * vt
                    b5a = b5a_t[vt % B5BUFS]
                    b5b = b5b_t[vt % B5BUFS]
                    nc.gpsimd.local_scatter(b5a[:], swT[:, 0:60],
                                            idxt[:, i0:i0 + 60],
                                            channels=P,
                                            num_elems=5 * BW64,
                                            num_idxs=60)
                    nc.gpsimd.local_scatter(b5b[:], swT[:, 60:108],
                                            idxt[:, i0 + 60:i0 + 108],
                                            channels=P,
                                            num_elems=4 * BW64,
                                            num_idxs=48)
                    b5s.append((b5a, b5b))

                tpsum_cm = tc.tile_pool(name="tpsum", bufs=1, space="PSUM")
                tpsum = tpsum_cm.__enter__()
                pst_t = [tpsum.tile([P, 108], F32, name=f"pst{i}")
                         for i in range(2)]

                def emit_b5_trans(vt, psy=None):
                    v0 = vt * P
                    gi = 0 if vt < 6 else (1 if vt < 12 else 2)
                    gv0 = SGROUPS[gi][2]
                    swsrc = swG[gi][:, v0 - gv0:v0 - gv0 + P]
                    if psy is None:
                        pst = pst_t[vt % 2][:, 0:108]
                    else:
                        pst = psy.tile([P, 512], F32, tag="psy",
                                       name=f"psty{vt}")[:, 0:108]
                    nc.tensor.matmul(pst, swsrc, permm[:],
                                     start=True, stop=True)
                    swT = swT_t[vt]
                    nc.scalar.activation(swT[:], pst, IDENT,
                                         bias=zb[:], scale=1.0)

                with tc.tile_pool(name="early", bufs=1) as early, \
                     tc.tile_pool(name="chunk", bufs=3) as chunk:
                    xsb = [early.tile([P, XL], BF, name=f"xsb{c}")
                           for c in range(CB)]
                    w1t = [early.tile([P, TAPS * 256], BF, name=f"w1t{c}")
                           for c in range(CB)]
                    w2t = [early.tile([P, TAPS * 256], BF, name=f"w2t{c}")
                           for c in range(CB)]
                    pwt8 = early.tile([P, TAPS * 96], F8, name="pwt8")
                    res1b = [early.tile([P, R1L], BF, name=f"res1b{c}")
                             for c in range(CB)]
                    zsb = early.tile([36, FW], BF, name="zsb")
                    res2f8 = early.tile([P, 2 * VPW], F8, name="res2f8")
                    fsb = early.tile([96, U], BF, name="fsb")
                    gsb = early.tile([96, U], BF, name="gsb")

                    # -------- input DMAs (merged, priority order) --------
                    # xsb[0] split so the first conv1 matmuls start early
                    XH = 1 + 22 * W
                    nc.sync.dma_start(out=xsb[0][:, 0:XH],
                                      in_=xsb_d.ap()[0][:, 0:XH])
                    nc.sync.dma_start(
                        out=w1t[0][:, 0:256], in_=w1t_d.ap()[0, 0])
                    nc.sync.dma_start(out=xsb[0][:, XH:XL],
                                      in_=xsb_d.ap()[0][:, XH:XL])
                    for t in range(1, TAPS):
                        nc.sync.dma_start(
                            out=w1t[0][:, t * 256:(t + 1) * 256],
                            in_=w1t_d.ap()[0, t])
                    # cb1 slab + weights on the idle gpsimd queue so they
                    # stream in parallel with the cb0 critical loads
                    nc.gpsimd.dma_start(out=xsb[1][:, 0:XH],
                                        in_=xsb_d.ap()[1][:, 0:XH])
                    nc.gpsimd.dma_start(out=xsb[1][:, XH:XL],
                                        in_=xsb_d.ap()[1][:, XH:XL])
                    for t in range(TAPS):
                        nc.gpsimd.dma_start(
                            out=w1t[1][:, t * 256:(t + 1) * 256],
                            in_=w1t_d.ap()[1, t])
                    nc.sync.dma_start(out=maskr[:], in_=maskr_d.ap())
                    for c in range(CB):
                        nc.sync.dma_start(out=b1[c][:], in_=b1_d.ap()[c])

                    def load_w(dram, tile_, c, mout):
                        src = _mk_src(dram,
                                      [[mout, P], [P * mout, TAPS], [1, mout]],
                                      c * TAPS * P * mout)
                        nc.sync.dma_start(out=tile_[:], in_=src)

                    for c in range(CB):
                        load_w(w2t_d, w2t[c], c, 256)
                        nc.sync.dma_start(out=b2[c][:], in_=b2_d.ap()[c])
                    for t in range(TAPS):
                        nc.sync.dma_start(
                            out=pwt8[:, t * 96:(t + 1) * 96],
                            in_=pw8_d.ap()[t])
                    nc.sync.dma_start(out=pb[:], in_=pb_d.ap())
                    nc.sync.dma_start(out=permm[:], in_=permm_d.ap())
                    nc.sync.dma_start(out=idxt[:], in_=idx_d.ap())
                    for c in range(CB):
                        nc.sync.dma_start(out=xs64[c][:], in_=xs64_d.ap()[c])
                    for c in range(CB):
                        nc.sync.dma_start(out=dwt[c][:], in_=dwt_d.ap()[c])

                    # zero staging planes (margins must read as 0)
                    nc.vector.memset(zsb[:], 0)
                    for h in range(3):
                        nc.sync.dma_start(out=fdramH[h].ap(), in_=zsb[:27, :])
                        nc.sync.dma_start(out=gdramH[h].ap(), in_=zsb[:, :])

                    # pre-zero so conv posts can write image columns only
                    # (pad columns stay zero; no end-of-phase memset barrier)
                    for c in range(CB):
                        nc.vector.memset(res1b[c][:], 0)
                        nc.vector.memset(res2b[c][:], 0)
                    nc.vector.memset(res2f8[:], 0)

                    # ---- phase 1: conv1 -> res1b ----
                    with tc.tile_pool(name="c1psum", bufs=6,
                                      space="PSUM") as cpsum:
                        for mb in range(CB):
                            psums = [cpsum.tile([P, 462], F32, tag="c1ps",
                                                name=f"c1ps_{mb}_{i}")
                                     for i in range(len(CONV1_CHUNKS))]
                            for cb in range(CB):
                                for t in range(TAPS):
                                    ky, kx = t // 3, t % 3
                                    lhsT = w1t[cb][:, t * 256 + mb * P:
                                                   t * 256 + mb * P + P]
                                    first = (cb == 0 and t == 0)
                                    last = (cb == CB - 1 and t == TAPS - 1)
                                    for ci, (c0, cr) in enumerate(CONV1_CHUNKS):
                                        o = 1 + (c0 + ky) * W + kx - 1
                                        nc.tensor.matmul(
                                            psums[ci][:, :cr * W], lhsT,
                                            xsb[cb][:, o:o + cr * W],
                                            start=first, stop=last)
                            for ci, (c0, cr) in enumerate(CONV1_CHUNKS):
                                n = cr * W
                                tmp = chunk.tile([P, 462], BF, tag="post")
                                nc.scalar.activation(tmp[:, :n],
                                                     psums[ci][:, :n],
                                                     RELU, bias=b1[mb][:],
                                                     scale=1.0)
                                tmp2 = chunk.tile([P, 462], BF, tag="post2")
                                nc.vector.tensor_tensor(
                                    tmp2[:, :n], tmp[:, :n],
                                    xsb[mb][:, 1 + (c0 + 1) * W:
                                            1 + (c0 + 1) * W + n], ADD)
                                mv = maskr[:, c0 + 1:c0 + 1 + cr, None] \
                                    .to_broadcast((P, cr, 64))
                                ov = res1b[mb][:, 1 + c0 * W:
                                               1 + (c0 + cr) * W]
                                nc.vector.tensor_tensor(
                                    ov.rearrange("p (r w) -> p r w",
                                                 w=W)[:, :, 1:65],
                                    tmp2[:, :n].rearrange(
                                        "p (r w) -> p r w", w=W)[:, :, 1:65],
                                    mv, MUL)

                    # ---- phase 2+3: conv2, offset conv, staging, scatters
                    def emit_conv2_chunk(mb, ps, ci):
                        e0, cr = CONV2_CHUNKS[ci]
                        for cb in range(CB):
                            for t in range(TAPS):
                                ky, kx = t // 3, t % 3
                                lhsT = w2t[cb][:, t * 256 + mb * P:
                                               t * 256 + mb * P + P]
                                first = (cb == 0 and t == 0)
                                last = (cb == CB - 1 and t == TAPS - 1)
                                o = 1 + (e0 + ky) * W + kx - 1
                                nc.tensor.matmul(
                                    ps[:, :cr * W], lhsT,
                                    res1b[cb][:, o:o + cr * W],
                                    start=first, stop=last)
                        n = cr * W
                        tmp = chunk.tile([P, 462], BF, tag="post")
                        nc.scalar.activation(tmp[:, :n], ps[:, :n],
                                             IDENT, bias=b2[mb][:], scale=1.0)
                        mv = maskr[:, e0 + 2:e0 + 2 + cr, None] \
                            .to_broadcast((P, cr, 64))
                        ov = res2b[mb][:, e0 * W:(e0 + cr) * W]
                        nc.vector.tensor_tensor(
                            ov.rearrange("p (r w) -> p r w", w=W)[:, :, 1:65],
                            tmp[:, :n].rearrange("p (r w) -> p r w",
                                                 w=W)[:, :, 1:65],
                            mv, MUL)
                        # fp8 copy for the offset conv rhs: second masked
                        # MUL straight from tmp (pads pre-zeroed at head)
                        of8 = res2f8[:, mb * VPW + e0 * W:
                                     mb * VPW + (e0 + cr) * W]
                        nc.vector.tensor_tensor(
                            of8.rearrange("p (r w) -> p r w", w=W)[:, :, 1:65],
                            tmp[:, :n].rearrange("p (r w) -> p r w",
                                                 w=W)[:, :, 1:65],
                            mv, MUL)

                    def emit_offs_chunk(opsum, m):
                        i0, cr = OFFS_CHUNKS[m]
                        n = cr * W
                        # fp8 DoubleRow: rhs windows must start 2B-aligned,
                        # so keep rhs at a fixed even base per tap and shift
                        # the psum write window by 1-kx instead (edge cols
                        # land on discarded pad positions).
                        ops = opsum.tile([48, 400], F32, tag="ops")
                        r3 = res2f8[:, :].rearrange("p (i v) -> p i v", i=2)
                        for t in range(TAPS):
                            ky, kx = t // 3, t % 3
                            lhsT = pwt8[:, t * 96:(t + 1) * 96].rearrange(
                                "p (i m) -> p i m", i=2)
                            r0 = (i0 + 1 + ky) * W
                            jj0 = 3 - kx
                            nc.tensor.matmul(
                                ops[:, jj0:jj0 + n + 1], lhsT,
                                r3[:, :, r0:r0 + n + 1],
                                start=(t == 0), stop=(t == TAPS - 1),
                                perf_mode=DR)
                        u0 = i0 * W
                        offs_c = chunk.tile([41, 396], F32, tag="offsc")
                        nc.scalar.activation(offs_c[:, :n], ops[0:41, 2:2 + n],
                                             IDENT, bias=pb[:],
                                             scale=1.0 / PW_SCALE)
                        offc_c = offs_c[32:41, :]
                        tmpa = chunk.tile([9, 396], BF, tag="tmpac")
                        tmpb = chunk.tile([9, 396], BF, tag="tmpbc")
                        # F rows: 0..8 relu(-t_r), 32.. 1-|t_r|, 64.. relu(t_r)
                        nc.vector.tensor_scalar(fsb[0:9, u0:u0 + n],
                                                offs_c[0:9, :n],
                                                0.0, -1.0, MIN, MUL)
                        nc.scalar.activation(tmpa[:, :n], offs_c[0:9, :n], ABS)
                        nc.vector.tensor_scalar(fsb[32:41, u0:u0 + n],
                                                tmpa[:, :n],
                                                -1.0, 1.0, MUL, ADD)
                        nc.vector.tensor_scalar(fsb[64:73, u0:u0 + n],
                                                offs_c[0:9, :n],
                                                0.0, 0.0, MAX, ADD)
                        # G rows: 0..8 relu(t_c) (jc=1), 32.. 1-|t_c| (jc=2),
                        # 64.. relu(-t_c) (jc=3)
                        nc.vector.tensor_scalar(gsb[0:9, u0:u0 + n],
                                                offc_c[:, :n],
                                                0.0, 0.0, MAX, ADD)
                        nc.scalar.activation(tmpb[:, :n], offc_c[:, :n], ABS)
                        nc.vector.tensor_scalar(gsb[32:41, u0:u0 + n],
                                                tmpb[:, :n],
                                                -1.0, 1.0, MUL, ADD)
                        nc.vector.tensor_scalar(gsb[64:73, u0:u0 + n],
                                                offc_c[:, :n],
                                                0.0, -1.0, MIN, MUL)

                    def emit_staging_group(gi):
                        v0, vn, ulo, uhi = SGROUPS[gi][2], SGROUPS[gi][3], \
                            SGROUPS[gi][4], SGROUPS[gi][5]
                        nu = uhi - ulo
                        # F skew: row 3n+a, col MARG+66*inr+inc+66a+u
                        for a in range(3):
                            dst = _mk_src(
                                fdramH[gi],
                                [[9 * FW + 66, 3], [3 * FW + 1, 3], [1, nu]],
                                a * FW + MARG + 66 * a + ulo)
                            nc.sync.dma_start(
                                out=dst, in_=fsb[32 * a:32 * a + 9, ulo:uhi])
                        # G: row 4n+jc, col MARG+66inr+inc+(3-jc)+u
                        # on sync: the scalar queue's activation backlog
                        # delayed these ~8us when they were scalar-issued
                        for jc in (1, 2, 3):
                            g0 = 32 * (jc - 1)
                            dst = _mk_src(
                                gdramH[gi],
                                [[12 * FW + 66, 3], [4 * FW + 1, 3], [1, nu]],
                                jc * FW + MARG + (3 - jc) + ulo)
                            nc.sync.dma_start(
                                out=dst, in_=gsb[g0:g0 + 9, ulo:uhi])
                        # reads: rows (a,n,b)/(a,n,c)-major, contiguous
                        rq = nc.gpsimd if gi == 0 else nc.sync
                        for a in range(3):
                            src = _mk_src(fdramH[gi],
                                          [[3 * FW, 9], [1, 4], [1, vn]],
                                          a * FW + MARG - 1 + v0)
                            rq.dma_start(out=prtG[gi][36 * a:36 * a + 36, :],
                                         in_=src)
                            src = _mk_src(gdramH[gi], [[FW, 36], [1, vn]],
                                          MARG + 2 - 66 * a + v0)
                            rq.dma_start(out=pctG[gi][36 * a:36 * a + 36, :],
                                         in_=src)
                        emit_swmul(gi)

                    NCH = len(OFFS_CHUNKS)
                    with tc.tile_pool(name="c2psum", bufs=3,
                                      space="PSUM") as c2psum, \
                         tc.tile_pool(name="opsum", bufs=1,
                                      space="PSUM") as opsum:
                        # alternate mb0/mb1 per row band: both channel
                        # blocks of each band finish together, so the
                        # offset conv / staging / scatter chain launches
                        # ~30us earlier than an mb-major sweep
                        for ci in range(len(CONV2_CHUNKS)):
                            for mb in range(CB):
                                ps = c2psum.tile([P, 462], F32, tag="c2ps",
                                                 name=f"c2ps_{mb}_{ci}")
                                emit_conv2_chunk(mb, ps, ci)
                            if ci >= 1:
                                emit_offs_chunk(opsum, ci - 1)
                            if ci == 2:
                                emit_staging_group(0)
                            elif ci == 3:
                                for vt in range(0, 6):
                                    emit_b5_trans(vt)
                            elif ci == 4:
                                emit_staging_group(1)
                            elif ci == 5:
                                for vt in range(6, 12):
                                    emit_b5_trans(vt)
                        emit_offs_chunk(opsum, NCH - 1)
                        emit_staging_group(2)
                        # scatters last on the Pool queue: all staged
                        # reads + sw multiplies are already queued ahead
                        for vt in range(0, 12):
                            emit_b5_scat(vt)

                # 'early' freed here; y / banded phases follow
                ypool_cm = tc.tile_pool(name="ypool", bufs=VT)
                ypool = ypool_cm.__enter__()

                def emit_y(psy, vt):
                    v0 = vt * P
                    y = ypool.tile([P, TAPS * 256], BF, name=f"y{vt}",
                                   tag="y")
                    for pc in range(5):
                        a0 = pc * 512
                        a1 = min(a0 + 512, TAPS * 256)
                        ps = psy.tile([P, 512], F32, tag="psy")
                        for cb in range(CB):
                            nc.tensor.matmul(ps[:, :a1 - a0],
                                             res2b[cb][:, v0:v0 + P],
                                             dwt[cb][:, a0:a1],
                                             start=(cb == 0),
                                             stop=(cb == CB - 1))
                        if pc in (0, 2, 4):
                            nc.vector.tensor_copy(y[:, a0:a1],
                                                  ps[:, :a1 - a0])
                        else:
                            nc.scalar.activation(y[:, a0:a1],
                                                 ps[:, :a1 - a0],
                                                 IDENT, bias=zb[:],
                                                 scale=1.0)
                    ys.append(y)

                # ---- phases 4+5: y-builds and a bank-split banded pass.
                # Low-u half (banks 0-1) runs right after y0-10 so the
                # vt0-5 b5 buffers free early, unblocking scat13-18 while
                # y11-18 still build; high-u half then runs un-paced.
                mms = []
                for vt in range(VT):
                    for n in range(TAPS):
                        w0 = W064[(vt, n)]
                        if w0 is None:
                            continue
                        lo = max(w0, 0)
                        hi = min(w0 + BW64, U64)
                        mms.append((vt, n, w0, lo, hi))
                lastj = {}
                for j, (vt, n, w0, lo, hi) in enumerate(mms):
                    for (sa, sb_) in _split_at_banks(lo, hi, 0):
                        lastj[sa // 512] = j
                psbs = {}

                def emit_seeds(bpsum, banks):
                    for mb in range(CB):
                        for k in banks:
                            psbs[(mb, k)] = bpsum.tile(
                                [P, 512], F32, name=f"psb_{mb}_{k}")
                            nc.tensor.matmul(
                                psbs[(mb, k)][:, :512],
                                ident[:],
                                xs64[mb][:, 512 * k:512 * (k + 1)],
                                start=True, stop=False,
                                skip_group_check=True)

                def emit_mms(banks, jlo=0, jhi=None):
                    if jhi is None:
                        jhi = len(mms)
                    for j in range(jlo, jhi):
                        (vt, n, w0, lo, hi) = mms[j]
                        segs = [s for s in _split_at_banks(lo, hi, 0)
                                if s[0] // 512 in banks]
                        if not segs:
                            continue
                        y = ys[vt]
                        b5a, b5b = b5s[vt]
                        for mb in range(CB):
                            lhsT = y[:, n * 256 + mb * P:
                                     n * 256 + mb * P + P]
                            for (sa, sb_) in segs:
                                bk = sa // 512
                                c0 = sa - bk * 512
                                if n < 5:
                                    rhs = b5a[:, n * BW64 + sa - w0:
                                              n * BW64 + sb_ - w0]
                                else:
                                    rhs = b5b[:,
                                              (n - 5) * BW64 + sa - w0:
                                              (n - 5) * BW64 + sb_ - w0]
                                nc.tensor.matmul(
                                    psbs[(mb, bk)][:, c0:c0 + sb_ - sa],
                                    lhsT, rhs, start=False,
                                    stop=(j == lastj[bk]),
                                    skip_group_check=True)

                def emit_out(banks):
                    for bank in banks:
                        for mb in range(CB):
                            outt = outt_t[2 * (bank % 2) + mb]
                            if mb == 0:
                                nc.scalar.activation(
                                    outt[:, :],
                                    psbs[(mb, bank)][:, :],
                                    IDENT, bias=zb[:], scale=1.0)
                            else:
                                nc.vector.tensor_copy(
                                    outt[:, :],
                                    psbs[(mb, bank)][:, :])
                            nc.sync.dma_start(
                                out=out_d.ap()[mb, :, 8 * bank:
                                               8 * bank + 8, :],
                                in_=outt[:, :].rearrange(
                                    "p (r w) -> p r w", w=64))

                with tc.tile_pool(name="psy", bufs=6,
                                  space="PSUM") as psy:
                    emit_y(psy, 0)
                    emit_y(psy, 1)
                    for vt in range(12, VT):
                        emit_b5_trans(vt)
                    emit_b5_scat(12)
                    for vt in range(2, VT):
                        emit_y(psy, vt)

                tpsum_cm.__exit__(None, None, None)

                with tc.tile_pool(name="bpsum", bufs=1,
                                  space="PSUM") as bpsum:
                    emit_seeds(bpsum, (0, 1, 2, 3))
                    jsplit = next(j for j, m in enumerate(mms)
                                  if m[0] >= 6)
                    emit_mms((0, 1, 2, 3), 0, jsplit)
                    for vt in range(B5BUFS, VT):
                        emit_b5_scat(vt)
                    emit_mms((0, 1, 2, 3), jsplit, None)
                    emit_out((0, 1, 2, 3))

                ypool_cm.__exit__(None, None, None)

    nc.finalize()
    return nc


def _pack_inputs(x, w1, b1, w2, b2, p_w, p_b, dw):
    """Build the 8 per-core input maps (numpy only)."""
    x = np.asarray(x, np.float32)

    def pack_w(w, mout):
        w = np.asarray(w, np.float32)
        out = np.empty((CB, TAPS, P, mout), bf16)
        for cb in range(CB):
            for t in range(TAPS):
                out[cb, t] = w[:, cb * P:(cb + 1) * P,
                               t // 3, t % 3].T.astype(bf16)
        return out

    w1t = pack_w(w1, 256)
    w2t = pack_w(w2, 256)
    # fp8 DoubleRow offset-conv weights: [TAPS, P, (cb, 41)] * PW_SCALE
    pwf = np.asarray(p_w, np.float32) * PW_SCALE
    pw8 = np.zeros((TAPS, P, 96), f8)
    for t in range(TAPS):
        for i in range(CB):
            blk = pwf[:, i * P:(i + 1) * P, t // 3, t % 3]  # [18, 128]
            pw8[t, :, i * 48 + 0:i * 48 + 9] = blk[0:9].T.astype(f8)
            pw8[t, :, i * 48 + 32:i * 48 + 41] = blk[9:18].T.astype(f8)
    dwt = np.empty((CB, P, TAPS * 256), bf16)
    dwf = np.asarray(dw, np.float32)
    for cb in range(CB):
        for t in range(TAPS):
            dwt[cb, :, t * 256:(t + 1) * 256] = \
                dwf[:, cb * P:(cb + 1) * P, t // 3, t % 3].T.astype(bf16)
    b1p = np.ascontiguousarray(np.asarray(b1, np.float32).reshape(CB, P, 1))
    b2p = np.ascontiguousarray(np.asarray(b2, np.float32).reshape(CB, P, 1))
    pb18 = np.asarray(p_b, np.float32).reshape(18)
    pbp = np.zeros((41, 1), np.float32)
    pbp[0:9, 0] = pb18[0:9]
    pbp[32:41, 0] = pb18[9:18]

    permm = np.zeros((108, 108), bf16)
    for n in range(9):
        for a in range(3):
            for b in range(4):
                permm[36 * a + 4 * n + b, 12 * n + 4 * a + b] = 1.0
    idx = np.full((P, VT * 108), -1, np.int16)
    for vt in range(VT):
        for n in range(TAPS):
            w0 = W064[(vt, n)]
            if w0 is None:
                continue
            zb_ = (n if n < 5 else n - 5) * BW64
            for a in range(3):
                for b in range(4):
                    l = 12 * n + 4 * a + b
                    for p in range(P):
                        u = _u64_map(vt, n, a, b, p)
                        if u is not None:
                            idx[p, 108 * vt + l] = zb_ + u - w0

    maps = []
    for core in range(N_CORES):
        b, half = core // 2, core % 2
        r0 = 32 * half
        slab = np.zeros((CB, P, TS, W), np.float32)
        g0, g1 = max(0, r0 - 4), min(64, r0 + 36)
        t0 = g0 - (r0 - 4)
        for cb in range(CB):
            slab[cb, :, t0:t0 + (g1 - g0), 1:65] = \
                x[b, cb * P:(cb + 1) * P, g0:g1, :]
        xsv = np.zeros((CB, P, XL), np.float32)
        xsv[:, :, 1:1 + TS * W] = slab.reshape(CB, P, TS * W)
        maskr = np.zeros((P, TS), bf16)
        valid = np.array([1.0 if 0 <= r0 - 4 + t < 64 else 0.0
                          for t in range(TS)], np.float32)
        maskr[:] = valid.astype(bf16)[None, :]
        xs64 = np.zeros((CB, P, U64), bf16)
        for cb in range(CB):
            xs64[cb] = slab[cb, :, 4:36, 1:65].reshape(P, U64).astype(bf16)
        maps.append({
            "xsb": xsv.astype(bf16), "maskr": maskr, "xs64": xs64,
            "w1t": w1t, "w2t": w2t, "pw8": pw8, "dwt": dwt,
            "b1": b1p, "b2": b2p, "pb": pbp, "permm": permm, "idx": idx,
        })
    return maps


def get_program():
    if "nc" not in _CACHE:
        _CACHE["nc"] = _build_program()
    return _CACHE["nc"]


def _ensure_ntff_hook():
    """The image's antenv lacks axon_hooks; inject a shim and register the
    NTFF profiling hook so trace=True works under axon."""
    import sys, types
    import antenv
    if "antenv.axon_hooks" in sys.modules:
        return
    mod = types.ModuleType("antenv.axon_hooks")
    mod._hook = None
    def set_axon_ntff_profile_hook(h):
        mod._hook = h
    def get_axon_ntff_profile_hook():
        return mod._hook
    mod.set_axon_ntff_profile_hook = set_axon_ntff_profile_hook
    mod.get_axon_ntff_profile_hook = get_axon_ntff_profile_hook
    sys.modules["antenv.axon_hooks"] = mod
    antenv.axon_hooks = mod
    try:
        from trn_agent_boot.trn_boot import _ntff_profile_via_ctypes
        hook = _ntff_profile_via_ctypes("/opt/axon/libaxon_pjrt.so")
        if hook is not None:
            set_axon_ntff_profile_hook(hook)
    except Exception as e:
        print("ntff hook setup failed:", e)


def kernel(x, w1, b1, w2, b2, p_w, p_b, dw):
    global LAST_RESULTS
    nc = get_program()
    maps = _pack_inputs(x, w1, b1, w2, b2, p_w, p_b, dw)
    trace = os.environ.get("DEHAZE_TRACE") == "1"
    if trace:
        _ensure_ntff_hook()
    res = run_bass_kernel_spmd(nc, maps, core_ids=list(range(N_CORES)),
                               trace=trace)
    LAST_RESULTS = res
    out = np.empty((4, 256, 64, 64), np.float32)
    for core in range(N_CORES):
        b, half = core // 2, core % 2
        o = res.results[core]["out"]  # [CB, P, RO, 64]
        out[b, :, 32 * half:32 * half + 32, :] = o.reshape(256, 32, 64)
    return out


# revision 53
# speedup vs baseline: 1.0234x; 1.0065x over previous
"""Trainium2 Bass kernel for a DehazeBlock:
    res1 = relu(conv3x3(x, w1) + b1) + x
    res2 = conv3x3(res1, w2) + b2
    out  = deform_conv(res2, p_w, p_b, dw) + x

Sharding: 8 cores = 4 batch x 2 H-halves (32 rows each, data-parallel,
communication-free; each core gets a zero-padded 40-row input slab).

Deform strategy (all-PE, no gathers): bilinear sampling at (base + t),
|t| < 1, decomposes over a 3-tap stencil (relu(-t), 1-|t|, relu(t)).
Fold the per-tap 256x256 channel mix first (y_n = DW_n @ res2), then
out[m,u] += sum_v y_n[v,m] * B_n[v,u] with B_n banded, built on-chip by
gpsimd local_scatter (host-precomputed index table, 64-stride output
space) from F*G weight planes staged through DRAM skew round-trips.

Schedule: conv1 -> conv2 with the offset conv (fp8 DoubleRow, K=256 per
instruction) interleaved; F/G staging, skewed reads, sw multiplies, swT
transposes and the serial gpsimd scatter chain all launch mid-conv2 so
the chain finishes long before the banded pass consumes it -> 19
y-tiles -> one full-width banded pass into 8 single-bank PSUM tiles
seeded with the x-residual by identity matmuls, each bank copied +
DMA'd out as its accumulation group closes.
"""

import os
import numpy as np
import ml_dtypes

import bass_rust
import concourse.bass as bass
import concourse.mybir as mybir
import concourse.tile as tile
from concourse import bacc
from concourse.bass_utils import run_bass_kernel_spmd
from concourse.masks import make_identity

bf16 = ml_dtypes.bfloat16
f8 = ml_dtypes.float8_e4m3
F32 = mybir.dt.float32
BF = mybir.dt.bfloat16
F8 = mybir.dt.float8e4
I16 = mybir.dt.int16

P = 128
CB = 2              # channel blocks (256 = 2*128)
W = 66              # padded row width
TS = 40             # x slab rows
R1 = 38             # res1 rows
R2 = 36             # res2 rows (= v rows)
RO = 32             # output rows per core
U = RO * W          # 2112 output pixel space
XL = TS * W + 2     # 2642 padded flat x row-span (+1 lead, +1 tail elem)
R1L = R1 * W + 2    # 2510
VT = 19             # v tiles
VPW = VT * P        # 2432
FW = 2720           # staging row width for F/G planes
MARG = 266
BW = 262            # banded-matrix u-window width per (tap, vtile)
TAPS = 9
N_CORES = 8
B5BUFS = 13         # b5 ring size; vtiles >= B5BUFS scattered during banded
PW_SCALE = 1024.0   # fp8 prescale for the tiny offset-conv weights
# 3-group staging pipeline: (vtile range, v0, vn, write ulo, write uhi)
SGROUPS = [(0, 6, 0, 768, 0, 12 * W),
           (6, 12, 768, 768, 6 * W, 24 * W),
           (12, 19, 1536, 896, 18 * W, 32 * W)]

CONV1_CHUNKS = [(0, 7), (7, 7), (14, 7), (21, 7), (28, 7), (35, 3)]
CONV2_CHUNKS = [(0, 6), (6, 6), (12, 6), (18, 6), (24, 6), (30, 6)]
OFFS_CHUNKS = [(0, 6), (6, 6), (12, 6), (18, 6), (24, 6), (30, 2)]
BW64 = 256          # banded window width in 64-stride output space
U64 = 2048          # 32 rows x 64 cols
DR = mybir.MatmulPerfMode.DoubleRow


def _u64_map(vt, n, a, b, p):
    """Output position (64-stride) hit by source row v=128vt+p, slot
    (n,a,b); None if it lands in a pad column / outside the image."""
    nr, ncc = n // 3 - 1, n % 3 - 1
    u66 = 128 * vt + p - 68 - 66 * nr - ncc - 66 * a + b
    if not (0 <= u66 < 2112 and 1 <= (u66 % 66) <= 64):
        return None
    return 64 * (u66 // 66) + (u66 % 66) - 1


def _w064_table():
    tab = {}
    for vt in range(VT):
        for n in range(TAPS):
            us = [u for a in range(3) for b in range(4) for p in range(P)
                  if (u := _u64_map(vt, n, a, b, p)) is not None]
            tab[(vt, n)] = min(us) if us else None
    return tab


_CACHE = {}
LAST_RESULTS = None
W064 = _w064_table()

MIN = mybir.AluOpType.min
MAX = mybir.AluOpType.max


def _mk_src(t, dims, off):
    s = t.ap().copy()
    s.ap = bass_rust.VecI64Pair(dims)
    s.offset = off
    return s


def _split_at_banks(lo, hi, base):
    """Split [lo, hi) (psum-tile-relative) at 512-elem bank boundaries."""
    segs = []
    a = lo
    while a < hi:
        b = min(hi, ((a - base) // 512 + 1) * 512 + base)
        segs.append((a, b))
        a = b
    return segs


def _build_program():
    nc = bacc.Bacc("TRN2", target_bir_lowering=False, debug=False,
                   num_devices=N_CORES)

    # ---------------- dram I/O ----------------
    xsb_d = nc.dram_tensor("xsb", [CB, P, XL], BF, kind="ExternalInput")
    maskr_d = nc.dram_tensor("maskr", [P, TS], BF, kind="ExternalInput")
    w1t_d = nc.dram_tensor("w1t", [CB, TAPS, P, 256], BF, kind="ExternalInput")
    w2t_d = nc.dram_tensor("w2t", [CB, TAPS, P, 256], BF, kind="ExternalInput")
    pw8_d = nc.dram_tensor("pw8", [TAPS, P, 96], F8, kind="ExternalInput")
    dwt_d = nc.dram_tensor("dwt", [CB, P, TAPS * 256], BF, kind="ExternalInput")
    b1_d = nc.dram_tensor("b1", [CB, P, 1], F32, kind="ExternalInput")
    b2_d = nc.dram_tensor("b2", [CB, P, 1], F32, kind="ExternalInput")
    pb_d = nc.dram_tensor("pb", [41, 1], F32, kind="ExternalInput")
    permm_d = nc.dram_tensor("permm", [108, 108], BF, kind="ExternalInput")
    xs64_d = nc.dram_tensor("xs64", [CB, P, U64], BF, kind="ExternalInput")
    idx_d = nc.dram_tensor("idx", [P, VT * 108], I16, kind="ExternalInput")
    out_d = nc.dram_tensor("out", [CB, P, RO, 64], F32, kind="ExternalOutput")

    fdramH = [nc.dram_tensor(f"fdram{h}", [27, FW], BF) for h in range(3)]
    gdramH = [nc.dram_tensor(f"gdram{h}", [36, FW], BF) for h in range(3)]

    RELU = mybir.ActivationFunctionType.Relu
    ABS = mybir.ActivationFunctionType.Abs
    IDENT = mybir.ActivationFunctionType.Identity
    MUL = mybir.AluOpType.mult
    ADD = mybir.AluOpType.add

    with tile.TileContext(nc) as tc:
      with tc.tile_pool(name="perm", bufs=1) as perm:
        ident = perm.tile([P, P], BF, name="ident")
        xs64 = [perm.tile([P, U64], BF, name=f"xs64_{c}") for c in range(CB)]
        idxt = perm.tile([P, VT * 108], I16, name="idxt")
        maskr = perm.tile([P, TS], BF, name="maskr")
        b1 = [perm.tile([P, 1], F32, name=f"b1{c}") for c in range(CB)]
        b2 = [perm.tile([P, 1], F32, name=f"b2{c}") for c in range(CB)]
        pb = perm.tile([41, 1], F32, name="pb")
        zb = perm.tile([P, 1], F32, name="zb")
        permm = perm.tile([108, 108], BF, name="permm")

        make_identity(nc, ident[:])
        nc.vector.memset(zb[:], 0)

        with tc.tile_pool(name="swpool", bufs=1) as swpool:
          ys, b5s, swTs = [], [], []
          swG = [swpool.tile([108, g[3]], BF, name=f"sw{i}")
                 for i, g in enumerate(SGROUPS)]
          dwt = [swpool.tile([P, TAPS * 256], BF, name=f"dwt{c}")
                 for c in range(CB)]
          res2b = [swpool.tile([P, VPW], BF, name=f"res2b{c}")
                   for c in range(CB)]
          with tc.tile_pool(name="prtpct", bufs=1) as prtpct:
            prtG = [prtpct.tile([108, g[3]], BF, name=f"prt{i}")
                    for i, g in enumerate(SGROUPS)]
            pctG = [prtpct.tile([108, g[3]], BF, name=f"pct{i}")
                    for i, g in enumerate(SGROUPS)]
            with tc.tile_pool(name="swtpool", bufs=1) as swtpool, \
                 tc.tile_pool(name="b5apool", bufs=1) as b5apool, \
                 tc.tile_pool(name="b5bpool", bufs=1) as b5bpool, \
                 tc.tile_pool(name="outpool", bufs=1) as outpool:

                # pre-create every ring tile so they sit BELOW the 'early'
                # region on the allocation stack (they outlive it)
                swT_t = [swtpool.tile([P, 108], BF, name=f"swT{i}")
                         for i in range(VT)]
                b5a_t = [b5apool.tile([P, 5 * BW64], BF, name=f"b5a{i}")
                         for i in range(B5BUFS)]
                b5b_t = [b5bpool.tile([P, 4 * BW64], BF, name=f"b5b{i}")
                         for i in range(B5BUFS)]
                outt_t = [outpool.tile([P, 512], F32, name=f"outt{i}")
                          for i in range(4)]

                def emit_swmul(gi):
                    # vector: Pool tensor_tensor is 2.5x slower and forces
                    # ucode lib swaps against the local_scatter chain
                    nc.vector.tensor_tensor(swG[gi][:], prtG[gi][:],
                                            pctG[gi][:], MUL)

                def emit_b5_scat(vt):
                    swT = swT_t[vt]
                    i0 = 108 * vt
                    b5a = b5a_t[vt % B5BUFS]
                    b5b = b5b_t[vt % B5BUFS]
                    nc.gpsimd.local_scatter(b5a[:], swT[:, 0:60],
                                            idxt[:, i0:i0 + 60],
                                            channels=P,
                                            num_elems=5 * BW64,
                                            num_idxs=60)
                    nc.gpsimd.local_scatter(b5b[:], swT[:, 60:108],
                                            idxt[:, i0 + 60:i0 + 108],
                                            channels=P,
                                            num_elems=4 * BW64,
                                            num_idxs=48)
                    b5s.append((b5a, b5b))

                tpsum_cm = tc.tile_pool(name="tpsum", bufs=1, space="PSUM")
                tpsum = tpsum_cm.__enter__()
                pst_t = [tpsum.tile([P, 108], F32, name=f"pst{i}")
                         for i in range(2)]

                def emit_b5_trans(vt, psy=None):
                    v0 = vt * P
                    gi = 0 if vt < 6 else (1 if vt < 12 else 2)
                    gv0 = SGROUPS[gi][2]
                    swsrc = swG[gi][:, v0 - gv0:v0 - gv0 + P]
                    if psy is None:
                        pst = pst_t[vt % 2][:, 0:108]
                    else:
                        pst = psy.tile([P, 512], F32, tag="psy",
                                       name=f"psty{vt}")[:, 0:108]
                    nc.tensor.matmul(pst, swsrc, permm[:],
                                     start=True, stop=True)
                    swT = swT_t[vt]
                    nc.scalar.activation(swT[:], pst, IDENT,
                                         bias=zb[:], scale=1.0)

                with tc.tile_pool(name="early", bufs=1) as early, \
                     tc.tile_pool(name="chunk", bufs=3) as chunk:
                    xsb = [early.tile([P, XL], BF, name=f"xsb{c}")
                           for c in range(CB)]
                    w1t = [early.tile([P, TAPS * 256], BF, name=f"w1t{c}")
                           for c in range(CB)]
                    w2t = [early.tile([P, TAPS * 256], BF, name=f"w2t{c}")
                           for c in range(CB)]
                    pwt8 = early.tile([P, TAPS * 96], F8, name="pwt8")
                    res1b = [early.tile([P, R1L], BF, name=f"res1b{c}")
                             for c in range(CB)]
                    zsb = early.tile([36, FW], BF, name="zsb")
                    res2f8 = early.tile([P, 2 * VPW], F8, name="res2f8")
                    fsb = early.tile([96, U], BF, name="fsb")
                    gsb = early.tile([96, U], BF, name="gsb")

                    # -------- input DMAs (merged, priority order) --------
                    # xsb[0] split so the first conv1 matmuls start early
                    XH = 1 + 22 * W
                    # first conv1 weights on the idle scalar queue so they
                    # stream in parallel with the x slab on sync
                    nc.scalar.dma_start(
                        out=w1t[0][:, 0:256], in_=w1t_d.ap()[0, 0])
                    nc.sync.dma_start(out=xsb[0][:, 0:XH],
                                      in_=xsb_d.ap()[0][:, 0:XH])
                    nc.scalar.dma_start(out=xsb[0][:, XH:XL],
                                        in_=xsb_d.ap()[0][:, XH:XL])
                    for t in range(1, 3):
                        nc.scalar.dma_start(
                            out=w1t[0][:, t * 256:(t + 1) * 256],
                            in_=w1t_d.ap()[0, t])
                    for t in range(3, TAPS):
                        nc.sync.dma_start(
                            out=w1t[0][:, t * 256:(t + 1) * 256],
                            in_=w1t_d.ap()[0, t])
                    # cb1 slab + weights on the idle gpsimd queue so they
                    # stream in parallel with the cb0 critical loads
                    nc.gpsimd.dma_start(out=xsb[1][:, 0:XH],
                                        in_=xsb_d.ap()[1][:, 0:XH])
                    nc.gpsimd.dma_start(out=xsb[1][:, XH:XL],
                                        in_=xsb_d.ap()[1][:, XH:XL])
                    for t in range(TAPS):
                        nc.gpsimd.dma_start(
                            out=w1t[1][:, t * 256:(t + 1) * 256],
                            in_=w1t_d.ap()[1, t])
                    nc.sync.dma_start(out=maskr[:], in_=maskr_d.ap())
                    for c in range(CB):
                        nc.sync.dma_start(out=b1[c][:], in_=b1_d.ap()[c])

                    def load_w(dram, tile_, c, mout):
                        src = _mk_src(dram,
                                      [[mout, P], [P * mout, TAPS], [1, mout]],
                                      c * TAPS * P * mout)
                        nc.sync.dma_start(out=tile_[:], in_=src)

                    for c in range(CB):
                        load_w(w2t_d, w2t[c], c, 256)
                        nc.sync.dma_start(out=b2[c][:], in_=b2_d.ap()[c])
                    for t in range(TAPS):
                        nc.sync.dma_start(
                            out=pwt8[:, t * 96:(t + 1) * 96],
                            in_=pw8_d.ap()[t])
                    nc.sync.dma_start(out=pb[:], in_=pb_d.ap())
                    nc.sync.dma_start(out=permm[:], in_=permm_d.ap())
                    nc.sync.dma_start(out=idxt[:], in_=idx_d.ap())
                    for c in range(CB):
                        nc.sync.dma_start(out=xs64[c][:], in_=xs64_d.ap()[c])
                    for c in range(CB):
                        nc.sync.dma_start(out=dwt[c][:], in_=dwt_d.ap()[c])

                    # zero staging planes (margins must read as 0)
                    nc.vector.memset(zsb[:], 0)
                    for h in range(3):
                        nc.sync.dma_start(out=fdramH[h].ap(), in_=zsb[:27, :])
                        nc.sync.dma_start(out=gdramH[h].ap(), in_=zsb[:, :])

                    # pre-zero so conv posts can write image columns only
                    # (pad columns stay zero; no end-of-phase memset barrier)
                    for c in range(CB):
                        nc.vector.memset(res1b[c][:], 0)
                        nc.vector.memset(res2b[c][:], 0)
                    nc.vector.memset(res2f8[:], 0)

                    # ---- phase 1: conv1 -> res1b ----
                    with tc.tile_pool(name="c1psum", bufs=6,
                                      space="PSUM") as cpsum:
                        for mb in range(CB):
                            psums = [cpsum.tile([P, 462], F32, tag="c1ps",
                                                name=f"c1ps_{mb}_{i}")
                                     for i in range(len(CONV1_CHUNKS))]
                            for cb in range(CB):
                                for t in range(TAPS):
                                    ky, kx = t // 3, t % 3
                                    lhsT = w1t[cb][:, t * 256 + mb * P:
                                                   t * 256 + mb * P + P]
                                    first = (cb == 0 and t == 0)
                                    last = (cb == CB - 1 and t == TAPS - 1)
                                    for ci, (c0, cr) in enumerate(CONV1_CHUNKS):
                                        o = 1 + (c0 + ky) * W + kx - 1
                                        nc.tensor.matmul(
                                            psums[ci][:, :cr * W], lhsT,
                                            xsb[cb][:, o:o + cr * W],
                                            start=first, stop=last)
                            for ci, (c0, cr) in enumerate(CONV1_CHUNKS):
                                n = cr * W
                                tmp = chunk.tile([P, 462], BF, tag="post")
                                nc.scalar.activation(tmp[:, :n],
                                                     psums[ci][:, :n],
                                                     RELU, bias=b1[mb][:],
                                                     scale=1.0)
                                tmp2 = chunk.tile([P, 462], BF, tag="post2")
                                nc.vector.tensor_tensor(
                                    tmp2[:, :n], tmp[:, :n],
                                    xsb[mb][:, 1 + (c0 + 1) * W:
                                            1 + (c0 + 1) * W + n], ADD)
                                mv = maskr[:, c0 + 1:c0 + 1 + cr, None] \
                                    .to_broadcast((P, cr, 64))
                                ov = res1b[mb][:, 1 + c0 * W:
                                               1 + (c0 + cr) * W]
                                nc.vector.tensor_tensor(
                                    ov.rearrange("p (r w) -> p r w",
                                                 w=W)[:, :, 1:65],
                                    tmp2[:, :n].rearrange(
                                        "p (r w) -> p r w", w=W)[:, :, 1:65],
                                    mv, MUL)

                    # ---- phase 2+3: conv2, offset conv, staging, scatters
                    def emit_conv2_chunk(mb, ps, ci):
                        e0, cr = CONV2_CHUNKS[ci]
                        for cb in range(CB):
                            for t in range(TAPS):
                                ky, kx = t // 3, t % 3
                                lhsT = w2t[cb][:, t * 256 + mb * P:
                                               t * 256 + mb * P + P]
                                first = (cb == 0 and t == 0)
                                last = (cb == CB - 1 and t == TAPS - 1)
                                o = 1 + (e0 + ky) * W + kx - 1
                                nc.tensor.matmul(
                                    ps[:, :cr * W], lhsT,
                                    res1b[cb][:, o:o + cr * W],
                                    start=first, stop=last)
                        n = cr * W
                        tmp = chunk.tile([P, 462], BF, tag="post")
                        nc.scalar.activation(tmp[:, :n], ps[:, :n],
                                             IDENT, bias=b2[mb][:], scale=1.0)
                        mv = maskr[:, e0 + 2:e0 + 2 + cr, None] \
                            .to_broadcast((P, cr, 64))
                        ov = res2b[mb][:, e0 * W:(e0 + cr) * W]
                        nc.vector.tensor_tensor(
                            ov.rearrange("p (r w) -> p r w", w=W)[:, :, 1:65],
                            tmp[:, :n].rearrange("p (r w) -> p r w",
                                                 w=W)[:, :, 1:65],
                            mv, MUL)
                        # fp8 copy for the offset conv rhs: second masked
                        # MUL straight from tmp (pads pre-zeroed at head)
                        of8 = res2f8[:, mb * VPW + e0 * W:
                                     mb * VPW + (e0 + cr) * W]
                        nc.vector.tensor_tensor(
                            of8.rearrange("p (r w) -> p r w", w=W)[:, :, 1:65],
                            tmp[:, :n].rearrange("p (r w) -> p r w",
                                                 w=W)[:, :, 1:65],
                            mv, MUL)

                    def emit_offs_chunk(opsum, m):
                        i0, cr = OFFS_CHUNKS[m]
                        n = cr * W
                        # fp8 DoubleRow: rhs windows must start 2B-aligned,
                        # so keep rhs at a fixed even base per tap and shift
                        # the psum write window by 1-kx instead (edge cols
                        # land on discarded pad positions).
                        ops = opsum.tile([48, 400], F32, tag="ops")
                        r3 = res2f8[:, :].rearrange("p (i v) -> p i v", i=2)
                        for t in range(TAPS):
                            ky, kx = t // 3, t % 3
                            lhsT = pwt8[:, t * 96:(t + 1) * 96].rearrange(
                                "p (i m) -> p i m", i=2)
                            r0 = (i0 + 1 + ky) * W
                            jj0 = 3 - kx
                            nc.tensor.matmul(
                                ops[:, jj0:jj0 + n + 1], lhsT,
                                r3[:, :, r0:r0 + n + 1],
                                start=(t == 0), stop=(t == TAPS - 1),
                                perf_mode=DR)
                        u0 = i0 * W
                        offs_c = chunk.tile([41, 396], F32, tag="offsc")
                        nc.scalar.activation(offs_c[:, :n], ops[0:41, 2:2 + n],
                                             IDENT, bias=pb[:],
                                             scale=1.0 / PW_SCALE)
                        offc_c = offs_c[32:41, :]
                        tmpa = chunk.tile([9, 396], BF, tag="tmpac")
                        tmpb = chunk.tile([9, 396], BF, tag="tmpbc")
                        # F rows: 0..8 relu(-t_r), 32.. 1-|t_r|, 64.. relu(t_r)
                        nc.vector.tensor_scalar(fsb[0:9, u0:u0 + n],
                                                offs_c[0:9, :n],
                                                0.0, -1.0, MIN, MUL)
                        nc.scalar.activation(tmpa[:, :n], offs_c[0:9, :n], ABS)
                        nc.vector.tensor_scalar(fsb[32:41, u0:u0 + n],
                                                tmpa[:, :n],
                                                -1.0, 1.0, MUL, ADD)
                        nc.vector.tensor_scalar(fsb[64:73, u0:u0 + n],
                                                offs_c[0:9, :n],
                                                0.0, 0.0, MAX, ADD)
                        # G rows: 0..8 relu(t_c) (jc=1), 32.. 1-|t_c| (jc=2),
                        # 64.. relu(-t_c) (jc=3)
                        nc.vector.tensor_scalar(gsb[0:9, u0:u0 + n],
                                                offc_c[:, :n],
                                                0.0, 0.0, MAX, ADD)
                        nc.scalar.activation(tmpb[:, :n], offc_c[:, :n], ABS)
                        nc.vector.tensor_scalar(gsb[32:41, u0:u0 + n],
                                                tmpb[:, :n],
                                                -1.0, 1.0, MUL, ADD)
                        nc.vector.tensor_scalar(gsb[64:73, u0:u0 + n],
                                                offc_c[:, :n],
                                                0.0, -1.0, MIN, MUL)

                    def emit_staging_group(gi):
                        v0, vn, ulo, uhi = SGROUPS[gi][2], SGROUPS[gi][3], \
                            SGROUPS[gi][4], SGROUPS[gi][5]
                        nu = uhi - ulo
                        # F skew: row 3n+a, col MARG+66*inr+inc+66a+u
                        for a in range(3):
                            dst = _mk_src(
                                fdramH[gi],
                                [[9 * FW + 66, 3], [3 * FW + 1, 3], [1, nu]],
                                a * FW + MARG + 66 * a + ulo)
                            nc.sync.dma_start(
                                out=dst, in_=fsb[32 * a:32 * a + 9, ulo:uhi])
                        # G: row 4n+jc, col MARG+66inr+inc+(3-jc)+u
                        # on sync: the scalar queue's activation backlog
                        # delayed these ~8us when they were scalar-issued
                        for jc in (1, 2, 3):
                            g0 = 32 * (jc - 1)
                            dst = _mk_src(
                                gdramH[gi],
                                [[12 * FW + 66, 3], [4 * FW + 1, 3], [1, nu]],
                                jc * FW + MARG + (3 - jc) + ulo)
                            nc.sync.dma_start(
                                out=dst, in_=gsb[g0:g0 + 9, ulo:uhi])
                        # reads: rows (a,n,b)/(a,n,c)-major, contiguous
                        rq = nc.gpsimd if gi == 0 else nc.sync
                        for a in range(3):
                            src = _mk_src(fdramH[gi],
                                          [[3 * FW, 9], [1, 4], [1, vn]],
                                          a * FW + MARG - 1 + v0)
                            rq.dma_start(out=prtG[gi][36 * a:36 * a + 36, :],
                                         in_=src)
                            src = _mk_src(gdramH[gi], [[FW, 36], [1, vn]],
                                          MARG + 2 - 66 * a + v0)
                            rq.dma_start(out=pctG[gi][36 * a:36 * a + 36, :],
                                         in_=src)
                        emit_swmul(gi)

                    NCH = len(OFFS_CHUNKS)
                    with tc.tile_pool(name="c2psum", bufs=3,
                                      space="PSUM") as c2psum, \
                         tc.tile_pool(name="opsum", bufs=1,
                                      space="PSUM") as opsum:
                        # alternate mb0/mb1 per row band: both channel
                        # blocks of each band finish together, so the
                        # offset conv / staging / scatter chain launches
                        # ~30us earlier than an mb-major sweep
                        for ci in range(len(CONV2_CHUNKS)):
                            for mb in range(CB):
                                ps = c2psum.tile([P, 462], F32, tag="c2ps",
                                                 name=f"c2ps_{mb}_{ci}")
                                emit_conv2_chunk(mb, ps, ci)
                            if ci >= 1:
                                emit_offs_chunk(opsum, ci - 1)
                            if ci == 2:
                                emit_staging_group(0)
                            elif ci == 3:
                                for vt in range(0, 6):
                                    emit_b5_trans(vt)
                            elif ci == 4:
                                emit_staging_group(1)
                            elif ci == 5:
                                for vt in range(6, 12):
                                    emit_b5_trans(vt)
                        emit_offs_chunk(opsum, NCH - 1)
                        emit_staging_group(2)
                        # scatters last on the Pool queue: all staged
                        # reads + sw multiplies are already queued ahead
                        for vt in range(0, 12):
                            emit_b5_scat(vt)

                # 'early' freed here; y / banded phases follow
                ypool_cm = tc.tile_pool(name="ypool", bufs=VT)
                ypool = ypool_cm.__enter__()

                def emit_y(psy, vt):
                    v0 = vt * P
                    y = ypool.tile([P, TAPS * 256], BF, name=f"y{vt}",
                                   tag="y")
                    for pc in range(5):
                        a0 = pc * 512
                        a1 = min(a0 + 512, TAPS * 256)
                        ps = psy.tile([P, 512], F32, tag="psy")
                        for cb in range(CB):
                            nc.tensor.matmul(ps[:, :a1 - a0],
                                             res2b[cb][:, v0:v0 + P],
                                             dwt[cb][:, a0:a1],
                                             start=(cb == 0),
                                             stop=(cb == CB - 1))
                        if pc in (0, 2, 4):
                            nc.vector.tensor_copy(y[:, a0:a1],
                                                  ps[:, :a1 - a0])
                        else:
                            nc.scalar.activation(y[:, a0:a1],
                                                 ps[:, :a1 - a0],
                                                 IDENT, bias=zb[:],
                                                 scale=1.0)
                    ys.append(y)

                # ---- phases 4+5: y-builds and a bank-split banded pass.
                # Low-u half (banks 0-1) runs right after y0-10 so the
                # vt0-5 b5 buffers free early, unblocking scat13-18 while
                # y11-18 still build; high-u half then runs un-paced.
                mms = []
                for vt in range(VT):
                    for n in range(TAPS):
                        w0 = W064[(vt, n)]
                        if w0 is None:
                            continue
                        lo = max(w0, 0)
                        hi = min(w0 + BW64, U64)
                        mms.append((vt, n, w0, lo, hi))
                lastj = {}
                for j, (vt, n, w0, lo, hi) in enumerate(mms):
                    for (sa, sb_) in _split_at_banks(lo, hi, 0):
                        lastj[sa // 512] = j
                psbs = {}

                def emit_seeds(bpsum, banks):
                    for mb in range(CB):
                        for k in banks:
                            psbs[(mb, k)] = bpsum.tile(
                                [P, 512], F32, name=f"psb_{mb}_{k}")
                            nc.tensor.matmul(
                                psbs[(mb, k)][:, :512],
                                ident[:],
                                xs64[mb][:, 512 * k:512 * (k + 1)],
                                start=True, stop=False,
                                skip_group_check=True)

                def emit_mms(banks, jlo=0, jhi=None):
                    if jhi is None:
                        jhi = len(mms)
                    for j in range(jlo, jhi):
                        (vt, n, w0, lo, hi) = mms[j]
                        segs = [s for s in _split_at_banks(lo, hi, 0)
                                if s[0] // 512 in banks]
                        if not segs:
                            continue
                        y = ys[vt]
                        b5a, b5b = b5s[vt]
                        for mb in range(CB):
                            lhsT = y[:, n * 256 + mb * P:
                                     n * 256 + mb * P + P]
                            for (sa, sb_) in segs:
                                bk = sa // 512
                                c0 = sa - bk * 512
                                if n < 5:
                                    rhs = b5a[:, n * BW64 + sa - w0:
                                              n * BW64 + sb_ - w0]
                                else:
                                    rhs = b5b[:,
                                              (n - 5) * BW64 + sa - w0:
                                              (n - 5) * BW64 + sb_ - w0]
                                nc.tensor.matmul(
                                    psbs[(mb, bk)][:, c0:c0 + sb_ - sa],
                                    lhsT, rhs, start=False,
                                    stop=(j == lastj[bk]),
                                    skip_group_check=True)

                def emit_out(banks):
                    for bank in banks:
                        for mb in range(CB):
                            outt = outt_t[2 * (bank % 2) + mb]
                            if mb == 0:
                                nc.scalar.activation(
                                    outt[:, :],
                                    psbs[(mb, bank)][:, :],
                                    IDENT, bias=zb[:], scale=1.0)
                            else:
                                nc.vector.tensor_copy(
                                    outt[:, :],
                                    psbs[(mb, bank)][:, :])
                            nc.sync.dma_start(
                                out=out_d.ap()[mb, :, 8 * bank:
                                               8 * bank + 8, :],
                                in_=outt[:, :].rearrange(
                                    "p (r w) -> p r w", w=64))

                with tc.tile_pool(name="psy", bufs=6,
                                  space="PSUM") as psy:
                    emit_y(psy, 0)
                    emit_y(psy, 1)
                    for vt in range(12, VT):
                        emit_b5_trans(vt)
                    emit_b5_scat(12)
                    for vt in range(2, VT):
                        emit_y(psy, vt)

                tpsum_cm.__exit__(None, None, None)

                with tc.tile_pool(name="bpsum", bufs=1,
                                  space="PSUM") as bpsum:
                    emit_seeds(bpsum, (0, 1, 2, 3))
                    jsplit = next(j for j, m in enumerate(mms)
                                  if m[0] >= 6)
                    emit_mms((0, 1, 2, 3), 0, jsplit)
                    for vt in range(B5BUFS, VT):
                        emit_b5_scat(vt)
                    emit_mms((0, 1, 2, 3), jsplit, None)
                    emit_out((0, 1, 2, 3))

                ypool_cm.__exit__(None, None, None)

    nc.finalize()
    return nc


def _pack_inputs(x, w1, b1, w2, b2, p_w, p_b, dw):
    """Build the 8 per-core input maps (numpy only)."""
    x = np.asarray(x, np.float32)

    def pack_w(w, mout):
        w = np.asarray(w, np.float32)
        out = np.empty((CB, TAPS, P, mout), bf16)
        for cb in range(CB):
            for t in range(TAPS):
                out[cb, t] = w[:, cb * P:(cb + 1) * P,
                               t // 3, t % 3].T.astype(bf16)
        return out

    w1t = pack_w(w1, 256)
    w2t = pack_w(w2, 256)
    # fp8 DoubleRow offset-conv weights: [TAPS, P, (cb, 41)] * PW_SCALE
    pwf = np.asarray(p_w, np.float32) * PW_SCALE
    pw8 = np.zeros((TAPS, P, 96), f8)
    for t in range(TAPS):
        for i in range(CB):
            blk = pwf[:, i * P:(i + 1) * P, t // 3, t % 3]  # [18, 128]
            pw8[t, :, i * 48 + 0:i * 48 + 9] = blk[0:9].T.astype(f8)
            pw8[t, :, i * 48 + 32:i * 48 + 41] = blk[9:18].T.astype(f8)
    dwt = np.empty((CB, P, TAPS * 256), bf16)
    dwf = np.asarray(dw, np.float32)
    for cb in range(CB):
        for t in range(TAPS):
            dwt[cb, :, t * 256:(t + 1) * 256] = \
                dwf[:, cb * P:(cb + 1) * P, t // 3, t % 3].T.astype(bf16)
    b1p = np.ascontiguousarray(np.asarray(b1, np.float32).reshape(CB, P, 1))
    b2p = np.ascontiguousarray(np.asarray(b2, np.float32).reshape(CB, P, 1))
    pb18 = np.asarray(p_b, np.float32).reshape(18)
    pbp = np.zeros((41, 1), np.float32)
    pbp[0:9, 0] = pb18[0:9]
    pbp[32:41, 0] = pb18[9:18]

    permm = np.zeros((108, 108), bf16)
    for n in range(9):
        for a in range(3):
            for b in range(4):
                permm[36 * a + 4 * n + b, 12 * n + 4 * a + b] = 1.0
    idx = np.full((P, VT * 108), -1, np.int16)
    for vt in range(VT):
        for n in range(TAPS):
            w0 = W064[(vt, n)]
            if w0 is None:
                continue
            zb_ = (n if n < 5 else n - 5) * BW64
            for a in range(3):
                for b in range(4):
                    l = 12 * n + 4 * a + b
                    for p in range(P):
                        u = _u64_map(vt, n, a, b, p)
                        if u is not None:
                            idx[p, 108 * vt + l] = zb_ + u - w0

    maps = []
    for core in range(N_CORES):
        b, half = core // 2, core % 2
        r0 = 32 * half
        slab = np.zeros((CB, P, TS, W), np.float32)
        g0, g1 = max(0, r0 - 4), min(64, r0 + 36)
        t0 = g0 - (r0 - 4)
        for cb in range(CB):
            slab[cb, :, t0:t0 + (g1 - g0), 1:65] = \
                x[b, cb * P:(cb + 1) * P, g0:g1, :]
        xsv = np.zeros((CB, P, XL), np.float32)
        xsv[:, :, 1:1 + TS * W] = slab.reshape(CB, P, TS * W)
        maskr = np.zeros((P, TS), bf16)
        valid = np.array([1.0 if 0 <= r0 - 4 + t < 64 else 0.0
                          for t in range(TS)], np.float32)
        maskr[:] = valid.astype(bf16)[None, :]
        xs64 = np.zeros((CB, P, U64), bf16)
        for cb in range(CB):
            xs64[cb] = slab[cb, :, 4:36, 1:65].reshape(P, U64).astype(bf16)
        maps.append({
            "xsb": xsv.astype(bf16), "maskr": maskr, "xs64": xs64,
            "w1t": w1t, "w2t": w2t, "pw8": pw8, "dwt": dwt,
            "b1": b1p, "b2": b2p, "pb": pbp, "permm": permm, "idx": idx,
        })
    return maps


def get_program():
    if "nc" not in _CACHE:
        _CACHE["nc"] = _build_program()
    return _CACHE["nc"]


def _ensure_ntff_hook():
    """The image's antenv lacks axon_hooks; inject a shim and register the
    NTFF profiling hook so trace=True works under axon."""
    import sys, types
    import antenv
    if "antenv.axon_hooks" in sys.modules:
        return
    mod = types.ModuleType("antenv.axon_hooks")
    mod._hook = None
    def set_axon_ntff_profile_hook(h):
        mod._hook = h
    def get_axon_ntff_profile_hook():
        return mod._hook
    mod.set_axon_ntff_profile_hook = set_axon_ntff_profile_hook
    mod.get_axon_ntff_profile_hook = get_axon_ntff_profile_hook
    sys.modules["antenv.axon_hooks"] = mod
    antenv.axon_hooks = mod
    try:
        from trn_agent_boot.trn_boot import _ntff_profile_via_ctypes
        hook = _ntff_profile_via_ctypes("/opt/axon/libaxon_pjrt.so")
        if hook is not None:
            set_axon_ntff_profile_hook(hook)
    except Exception as e:
        print("ntff hook setup failed:", e)


def kernel(x, w1, b1, w2, b2, p_w, p_b, dw):
    global LAST_RESULTS
    nc = get_program()
    maps = _pack_inputs(x, w1, b1, w2, b2, p_w, p_b, dw)
    trace = os.environ.get("DEHAZE_TRACE") == "1"
    if trace:
        _ensure_ntff_hook()
    res = run_bass_kernel_spmd(nc, maps, core_ids=list(range(N_CORES)),
                               trace=trace)
    LAST_RESULTS = res
    out = np.empty((4, 256, 64, 64), np.float32)
    for core in range(N_CORES):
        b, half = core // 2, core % 2
        o = res.results[core]["out"]  # [CB, P, RO, 64]
        out[b, :, 32 * half:32 * half + 32, :] = o.reshape(256, 32, 64)
    return out
